# revision 1
# baseline (speedup 1.0000x reference)
"""Trainium2 Bass kernel: distributed GIN graph encoder on 8 NeuronCores.

v4: sub-phase aggregation is a single-stage gather (per source-bucket windows,
(bucket x 512-dst-group) cells) + on-chip one-hot S generation on the Scalar
engine (S = relu(1 - (iota - d)^2)) + SBUF bf16 accumulator. Initial atom
embeddings precomputed on host. Glob phase keeps the per-tile DRAM-S scheme.
"""

import numpy as np
import ml_dtypes

BF = ml_dtypes.bfloat16
F16 = np.float16

DEF_CFG = dict(
    W=8, H=128, L=4,
    n_sub=30000, S_sub=30080,     # per-core real/padded sub rows
    n_glob=3750, S_glob=3840,
    VA=128, G=300, TEMP=0.5, BN_EPS=1e-5,
    GRPW=256,                     # dst psum group width (sub)
    CALLCH=32,                    # chunks per gather call (sub)
    GRP=16,                       # glob: chunks per gather call
)


def _pack16(arr):
    """idx array (n,) int -> [128, n/16] int16 tile content (pos i -> [i%16, i//16])."""
    a = np.asarray(arr, np.int16)
    assert len(a) % 16 == 0
    t = a.reshape(-1, 16).T
    return np.tile(t, (8, 1))


def _pad128(n):
    return (n + 127) // 128 * 128


def build_plan(cfg, x, edge_index, sub_node_map, sub_edge_index, root_idx,
               target_batch, batch):
    W, H = cfg["W"], cfg["H"]
    n_sub, S_sub = cfg["n_sub"], cfg["S_sub"]
    n_glob, S_glob = cfg["n_glob"], cfg["S_glob"]
    GRPW = cfg["GRPW"]
    plan = {"cfg": cfg}
    N = n_glob * W

    # ---------- sub phase edge plan (single-stage, shared budgets) ----------
    src, dst = np.asarray(sub_edge_index[0]), np.asarray(sub_edge_index[1])
    owner = dst // n_sub
    dst_local = dst % n_sub
    src_row = (src // n_sub) * S_sub + (src % n_sub)   # row in padded replica
    ngrp = (S_sub + GRPW - 1) // GRPW

    # group edges per (core, bucket, dst-group); shared budget = max over cores
    per_core = []
    cnt = np.zeros((W, W, ngrp), np.int64)   # [core, bucket, group]
    for c in range(W):
        m = owner == c
        sc, dl = src_row[m], dst_local[m]
        b = sc // S_sub
        q = sc % S_sub
        g = dl // GRPW
        order = np.lexsort((q, g, b))
        b, q, dl, g = b[order], q[order], dl[order], g[order]
        per_core.append((b, q, dl, g))
        np.add.at(cnt[c], (b, g), 1)
    budget = _pad128(cnt.max(axis=0))        # [bucket, group] slots (multiple of 128)
    budget = np.maximum(budget, 128)
    GH = (ngrp + 1) // 2
    if (GH * GRPW) % 512:
        GH += 1          # keep the acc0/acc1 split 512-aligned for MLP groups
    nslots = int(budget.sum())
    nchunks = int((budget // 128).sum())
    # shared chunk list + calls, ordered (half, bucket, group)
    cell_off2 = np.zeros((W, ngrp), np.int64)
    off = 0
    chunks = []   # per chunk: [(g, first, stop, gfirst)]
    calls = []    # (bucket, chunk_start, nchunks, end_of_half0)
    for hh in range(2):
        glo, ghi = hh * GH, min((hh + 1) * GH, ngrp)
        for bb in range(W):
            c0 = len(chunks)
            for g in range(glo, ghi):
                cell_off2[bb, g] = off
                off += int(budget[bb, g])
                nch = int(budget[bb, g]) // 128
                for j in range(nch):
                    chunks.append([(g, j == 0, j == nch - 1, bb == 0)])
            done = 0
            nb = len(chunks) - c0
            while done < nb:
                k = min(cfg["CALLCH"], nb - done)
                done += k
                calls.append((bb, c0 + done - k, k,
                              hh == 0 and bb == W - 1 and done >= nb))
    assert off == nslots
    # per-core slot data; drel stores NEGATED dst_rel (ACT Square bias), pad -> +1
    sub_qidx, sub_drel = [], []
    for c in range(W):
        b, q, dl, g = per_core[c]
        qpad = np.zeros(nslots, np.int64)
        dpad = np.full(nslots, -1, np.int64)
        cell = b * ngrp + g
        uniq, start_idx, counts = np.unique(cell, return_index=True, return_counts=True)
        for u, s0, k in zip(uniq, start_idx, counts):
            sl = cell_off2[u // ngrp, u % ngrp] + np.arange(k)
            qpad[sl] = q[s0:s0 + k]
            dpad[sl] = dl[s0:s0 + k] - (u % ngrp) * GRPW
        sub_qidx.append(_pack16(qpad))
        sub_drel.append((-dpad.reshape(-1, 128).T).astype(np.float32))  # [128, nchunks]
    plan["sub_nslots"] = nslots
    plan["sub_nops"] = nchunks
    plan["sub_qidx"] = sub_qidx
    plan["sub_drel"] = sub_drel
    plan["sub_chunks"] = chunks
    plan["sub_calls"] = calls
    plan["sub_ngrp"] = ngrp
    plan["sub_GH"] = GH

    # ---------- glob phase edge plan ----------
    gsrc, gdst = np.asarray(edge_index[0]), np.asarray(edge_index[1])
    gowner = gdst // n_glob
    gdst_local = gdst % n_glob
    gsrc_row = (gsrc // n_glob) * S_glob + (gsrc % n_glob)
    n_tiles_glob = S_glob // 128
    per_core_g = []
    CtG = np.zeros(n_tiles_glob, np.int64)
    for c in range(W):
        m = gowner == c
        sc, dl = gsrc_row[m], gdst_local[m]
        order = np.argsort(dl, kind="stable")
        sc, dl = sc[order], dl[order]
        per_core_g.append((sc, dl))
        gcnt = np.bincount(dl // 128, minlength=n_tiles_glob)
        CtG = np.maximum(CtG, (gcnt + 127) // 128)
    CtG = np.maximum(CtG, 1)
    plan["glob_Ct"] = CtG
    TOTCHG = int(CtG.sum())
    plan["glob_TOTCH"] = TOTCHG
    tile_slot_off_g = np.zeros(n_tiles_glob + 1, np.int64)
    tile_slot_off_g[1:] = np.cumsum(CtG * 128)
    gg_idx_cores, Sg_cores = [], []
    for c in range(W):
        sc, dl = per_core_g[c]
        g2 = np.zeros(TOTCHG * 128, np.int64)
        S = np.zeros((TOTCHG * 128, 128), BF)
        tl = dl // 128
        for t in range(n_tiles_glob):
            mt = tl == t
            k = int(mt.sum())
            slots = tile_slot_off_g[t] + np.arange(k)
            g2[slots] = sc[mt]
            S[slots, dl[mt] % 128] = BF(1.0)
        gg_idx_cores.append(_pack16(g2))
        Sg_cores.append(S)
    plan["glob_g2_idx"] = gg_idx_cores
    plan["glob_S"] = [S.reshape(TOTCHG, 128, 128).transpose(1, 0, 2).reshape(128, TOTCHG * 128)
                      for S in Sg_cores]

    # ---------- atom encode (host) ----------
    plan["aid_ids"] = np.asarray(x)[np.asarray(sub_node_map)]

    # ---------- phase boundary (roots) ----------
    tb_arr = np.asarray(target_batch)
    ri = np.asarray(root_idx)
    order = np.argsort(tb_arr, kind="stable")
    assert (np.bincount(tb_arr, minlength=N) == 2).all(), "need exactly 2 roots/node"
    r_sorted = ri[order].reshape(N, 2)
    lp_order = order.reshape(N, 2)
    plan["r0"], plan["r4"], plan["lp_sel"] = [], [], []
    for c in range(W):
        r = r_sorted[c * n_glob:(c + 1) * n_glob]
        lo = c * n_sub
        assert ((r >= lo) & (r < lo + n_sub)).all(), "roots must be core-local"
        r0 = np.zeros(S_glob, np.int64)
        r4 = np.zeros(S_glob, np.int64)
        r0[:n_glob] = r[:, 0] - lo
        r4[:n_glob] = r[:, 1] - lo
        plan["r0"].append(_pack16(r0))
        plan["r4"].append(_pack16(r4))
        plan["lp_sel"].append(lp_order[c * n_glob:(c + 1) * n_glob])

    # ---------- readout ----------
    b_arr = np.asarray(batch)
    plan["Sg"] = []
    for c in range(W):
        Srd = np.zeros((S_glob, cfg["G"]), BF)
        ids = b_arr[c * n_glob:(c + 1) * n_glob]
        Srd[np.arange(n_glob), ids] = BF(1.0)
        nt = S_glob // 128
        plan["Sg"].append(Srd.reshape(nt, 128, cfg["G"]).transpose(1, 0, 2).reshape(128, nt * cfg["G"]))
    return plan


def _install_queue_aware_lanes():
    """Make Tile's DMASW lane assignment queue-aware: lane = queue*2 + rr."""
    import concourse.tile_sem_assignment as tsa
    if getattr(tsa, "_qaware_installed", False):
        return
    orig = tsa.TileClockTick._assign_tick
    import concourse.mybir as mb

    def patched(self, inst):
        qn = getattr(inst, "queue_num", None)
        if (qn is not None and inst.engine == mb.EngineType.Pool
                and isinstance(inst, tsa.DMAInst)
                and self.swdge_sem_count == 8):
            rr_map = getattr(self, "_q_rr", None)
            if rr_map is None:
                rr_map = self._q_rr = {}
            sub = rr_map.get(qn, 0)
            rr_map[qn] = (sub + 1) % 2
            lane = qn * 2 + sub
            save = self.next_sw_dma_idx
            self.next_sw_dma_idx = lane
            try:
                return orig(self, inst)
            finally:
                self.next_sw_dma_idx = save
        return orig(self, inst)

    tsa.TileClockTick._assign_tick = patched
    tsa._qaware_installed = True


def build_graph(plan):
    from concourse import bass, mybir, bacc
    import concourse.tile as tile

    cfg = plan["cfg"]
    W, H, L = cfg["W"], cfg["H"], cfg["L"]
    n_sub, S_sub = cfg["n_sub"], cfg["S_sub"]
    n_glob, S_glob = cfg["n_glob"], cfg["S_glob"]
    G = cfg["G"]
    GRPW = cfg["GRPW"]
    BF16 = mybir.dt.bfloat16
    FP16 = mybir.dt.float16
    F32 = mybir.dt.float32
    I16 = mybir.dt.int16
    AF = mybir.ActivationFunctionType
    OP = mybir.AluOpType
    GRP = cfg["GRP"]
    ngrp = plan["sub_ngrp"]

    _install_queue_aware_lanes()
    nc = bacc.Bacc("TRN2", target_bir_lowering=False, debug=False, num_devices=W,
                   num_swdge_queues=4)

    def inp(name, shape, dt):
        return nc.dram_tensor(name, shape, dt, kind="ExternalInput")

    TOTCHG = plan["glob_TOTCH"]
    nslots, nops = plan["sub_nslots"], plan["sub_nops"]
    t_qidx = inp("qidx", [128, nslots // 16], I16)
    t_drel = inp("drel", [128, nops], F32)
    t_iota = inp("iota16", [128, GRPW], FP16)
    t_gg = inp("ggidx", [128, TOTCHG * 8], I16)
    t_h0 = inp("h0", [S_sub, H], BF16)
    t_r0 = inp("r0idx", [128, S_glob // 16], I16)
    t_r4 = inp("r4idx", [128, S_glob // 16], I16)
    t_Sglob = inp("Sglob", [128, TOTCHG * 128], BF16)
    t_Srd = inp("Srd", [128, (S_glob // 128) * G], BF16)
    t_idn_bf = inp("idnbf", [128, 128], BF16)
    t_idn_f = inp("idnf", [128, 128], F32)
    t_atom = inp("atom", [cfg["VA"], H], BF16)
    t_W1s = inp("W1s", [L, H, H], BF16)
    t_W2s = inp("W2s", [L, H, H], BF16)
    t_W1g = inp("W1g", [L, H, H], BF16)
    t_W2g = inp("W2g", [L, H, H], BF16)
    t_vecs = inp("vecs", [128, 10 * L], F32)
    t_lp = inp("lp", [S_glob, 2], F32)
    t_out = nc.dram_tensor("out", [G, H], F32, kind="ExternalOutput")

    rep_sub = nc.dram_tensor("rep_sub", [W * S_sub, H], BF16, addr_space="Shared")
    rep_glob = nc.dram_tensor("rep_glob", [W * S_glob, H], BF16, addr_space="Shared")
    hown_sub = nc.dram_tensor("hown_sub", [S_sub, H], BF16)
    hown_glob = nc.dram_tensor("hown_glob", [S_glob, H], BF16)
    ar_in = nc.dram_tensor("ar_in", [128, 2], F32)
    ar_out = nc.dram_tensor("ar_out", [128, 2], F32, addr_space="Shared")
    rd_in = nc.dram_tensor("rd_in", [128, G], F32)
    rd_out = nc.dram_tensor("rd_out", [128, G], F32, addr_space="Shared")

    RG = [list(range(W))]
    _qrr = [0]

    def nextq():
        q = (_qrr[0] % 8) // 2
        _qrr[0] += 1
        return q

    CtG = plan["glob_Ct"]
    chunks_meta = plan["sub_chunks"]
    calls_meta = plan["sub_calls"]
    GH = plan["sub_GH"]
    SPLITC = GH * GRPW

    with tile.TileContext(nc) as tc:
        with (
            tc.tile_pool(name="const", bufs=1) as constp,
            tc.tile_pool(name="idx", bufs=1) as idxp,
            tc.tile_pool(name="seg", bufs=2) as segp,
            tc.tile_pool(name="xs", bufs=3) as xsp,
            tc.tile_pool(name="sgen", bufs=3) as sgp,
            tc.tile_pool(name="zz", bufs=3) as zzp,
            tc.tile_pool(name="res", bufs=1) as resp,
            tc.tile_pool(name="small", bufs=2) as smp,
            tc.tile_pool(name="stg", bufs=2) as stgp,
            tc.tile_pool(name="psA", bufs=2, space="PSUM") as psA,
            tc.tile_pool(name="psM", bufs=1, space="PSUM") as psM,
            tc.tile_pool(name="psT", bufs=2, space="PSUM") as psT,
        ):
            # ---- constants resident ----
            vecs = constp.tile([128, 10 * L], F32)
            nc.sync.dma_start(vecs[:], t_vecs[:])
            W1s = constp.tile([128, L * H], BF16)
            W2s = constp.tile([128, L * H], BF16)
            W1g = constp.tile([128, L * H], BF16)
            W2g = constp.tile([128, L * H], BF16)
            for l in range(L):
                nc.sync.dma_start(W1s[:, l * H:(l + 1) * H], t_W1s[l])
                nc.sync.dma_start(W2s[:, l * H:(l + 1) * H], t_W2s[l])
                nc.sync.dma_start(W1g[:, l * H:(l + 1) * H], t_W1g[l])
                nc.sync.dma_start(W2g[:, l * H:(l + 1) * H], t_W2g[l])
            idn = constp.tile([128, 128], BF16, tag="idn")
            nc.sync.dma_start(idn[:], t_idn_bf[:])
            qidx_sb = constp.tile([128, nslots // 16], I16, tag="qidx")
            nc.sync.dma_start(qidx_sb[:], t_qidx[:])
            drel_sb = constp.tile([128, nops], F32, tag="drel")
            nc.sync.dma_start(drel_sb[:], t_drel[:])
            iota_sb = constp.tile([128, GRPW], FP16, tag="iota")
            nc.sync.dma_start(iota_sb[:], t_iota[:])
            gg_sb = constp.tile([128, TOTCHG * 8], I16, tag="ggix")
            nc.sync.dma_start(gg_sb[:], t_gg[:])

            def vcol(phase, l, j):
                return vecs[:, (phase * 5 * L + l * 5 + j):(phase * 5 * L + l * 5 + j) + 1]

            # ---- atom encode (host-precomputed h0) -> hown_sub ----
            nc.sync.dma_start(hown_sub.ap(), t_h0.ap())

            # shared BN + normalize tail --------------------------------
            def bn_tail(phase, l, z2, stats, Sp, n_real, hown, n_tiles):
                if callable(z2):
                    zread = z2
                else:
                    zt = z2
                    def zread(g):
                        return zt, g * 512
                ngr = (n_real + 511) // 512
                mv = smp.tile([128, 2], F32, tag="mv")
                nc.vector.bn_aggr(mv[:], stats[:, :ngr * 6])
                sin = smp.tile([128, 2], F32, tag="sin")
                nc.vector.tensor_tensor(sin[:, 1:2], mv[:, 0:1], mv[:, 0:1], op=OP.mult)
                nc.vector.tensor_tensor(sin[:, 1:2], sin[:, 1:2], mv[:, 1:2], op=OP.add)
                nc.vector.tensor_copy(sin[:, 0:1], mv[:, 0:1])
                nc.sync.dma_start(ar_in[:], sin[:])
                nc.gpsimd.collective_compute(
                    "AllReduce", OP.add, RG, [ar_in.ap().opt()], [ar_out.ap().opt()])
                sg = smp.tile([128, 2], F32, tag="sg")
                nc.sync.dma_start(sg[:], ar_out[:])
                mu = smp.tile([128, 4], F32, tag="mu")
                nc.scalar.mul(mu[:, 0:1], sg[:, 0:1], 1.0 / W)
                nc.scalar.mul(mu[:, 1:2], sg[:, 1:2], 1.0 / W)
                nc.vector.tensor_tensor(mu[:, 2:3], mu[:, 0:1], mu[:, 0:1], op=OP.mult)
                nc.vector.tensor_tensor(mu[:, 1:2], mu[:, 1:2], mu[:, 2:3], op=OP.subtract)
                nc.vector.tensor_scalar(mu[:, 1:2], mu[:, 1:2], float(cfg["BN_EPS"]), None,
                                        op0=OP.add)
                nc.scalar.activation(mu[:, 1:2], mu[:, 1:2], AF.Sqrt, bias=0.0, scale=1.0)
                nc.vector.reciprocal(mu[:, 1:2], mu[:, 1:2])
                nc.vector.tensor_tensor(mu[:, 2:3], vcol(phase, l, 2), mu[:, 1:2], op=OP.mult)
                nc.vector.tensor_tensor(mu[:, 3:4], mu[:, 0:1], mu[:, 2:3], op=OP.mult)
                nc.vector.tensor_tensor(mu[:, 3:4], vcol(phase, l, 3), mu[:, 3:4], op=OP.subtract)

                stgt = stgp.tile([128, 16, 128], BF16, tag="stg")
                stg_fill = 0
                stg_t0 = 0
                for blk in range((Sp + 1023) // 1024):
                    base = blk * 1024
                    cols = min(1024, Sp - base)
                    hg = zzp.tile([128, 1024], BF16, tag="hg2")
                    nc.sync.dma_start(hg[:, :cols], hown[base:base + cols, :],
                                      transpose=True)
                    hn = zzp.tile([128, 1024], BF16, tag="hn")
                    for s in range(0, cols, 512):
                        sc = min(512, cols - s)
                        zt_, co_ = zread((base + s) // 512)
                        nc.vector.tensor_scalar(hn[:, s:s + sc], zt_[:, co_:co_ + sc],
                                                mu[:, 2:3], mu[:, 3:4],
                                                op0=OP.mult, op1=OP.add)
                    nc.vector.tensor_tensor(hn[:, :cols], hn[:, :cols], hg[:, :cols], op=OP.add)
                    for q in range(cols // 128):
                        t = base // 128 + q
                        pt = psT.tile([128, 128], BF16, tag="tr")
                        nc.tensor.transpose(pt[:], hn[:, q * 128:(q + 1) * 128], idn[:])
                        nc.vector.tensor_copy(stgt[:, stg_fill, :], pt[:])
                        stg_fill += 1
                        if stg_fill == 16 or t == n_tiles - 1:
                            nc.sync.dma_start(
                                hown.ap().rearrange("(c p) h -> p c h", p=128)[:, stg_t0:stg_t0 + stg_fill, :],
                                stgt[:, :stg_fill, :])
                            stg_t0 += stg_fill
                            stg_fill = 0
                            if t != n_tiles - 1:
                                stgt = stgp.tile([128, 16, 128], BF16, tag="stg")

            # ---- sub phase GIN layer ----
            def gin_sub(l):
                nc.gpsimd.collective_compute(
                    "AllGather", OP.bypass, RG, [hown_sub.ap().opt()], [rep_sub.ap().opt()])
                acc0 = resp.tile([128, SPLITC], BF16, tag="acc0")
                acc1 = resp.tile([128, S_sub - SPLITC], BF16, tag="acc1")
                stats = smp.tile([128, 64 * 6], F32, tag="stats")

                def acc_at(g):
                    if g < GH:
                        return acc0, g * GRPW
                    return acc1, g * GRPW - SPLITC

                def acc512(g):
                    # 512-col MLP-group accessor (GRPW divides 512)
                    co = g * 512
                    if co < SPLITC:
                        return acc0, co
                    return acc1, co - SPLITC

                def epilogue(g0, g1):
                    g = g0
                    while g < g1:
                        nblk = min(2, g1 - g)
                        bcols = min(nblk * 512, S_sub - g * 512)
                        bat, bco = acc512(g)
                        hg = zzp.tile([128, 1024], BF16, tag="hg")
                        nc.sync.dma_start(hg[:, :bcols],
                                          hown_sub[g * 512:g * 512 + bcols, :],
                                          transpose=True)
                        zg = zzp.tile([128, 1024], BF16, tag="zg")
                        nc.vector.scalar_tensor_tensor(
                            zg[:, :bcols], hg[:, :bcols], vcol(0, l, 4),
                            bat[:, bco:bco + bcols], op0=OP.mult, op1=OP.add)
                        for s in range(nblk):
                            gg = g + s
                            cols = min(512, S_sub - gg * 512)
                            at, co = acc512(gg)
                            pm = psM.tile([128, 512], F32, tag="m1")
                            nc.tensor.matmul(pm[:, :cols], W1s[:, l * H:(l + 1) * H],
                                             zg[:, s * 512:s * 512 + cols],
                                             start=True, stop=True)
                            z1 = zzp.tile([128, 512], BF16, tag="z1")
                            nc.scalar.activation(z1[:, :cols], pm[:, :cols], AF.Relu,
                                                 bias=vcol(0, l, 0), scale=1.0)
                            pm2 = psM.tile([128, 512], F32, tag="m2")
                            nc.tensor.matmul(pm2[:, :cols], W2s[:, l * H:(l + 1) * H],
                                             z1[:, :cols], start=True, stop=True)
                            nc.vector.tensor_scalar(at[:, co:co + cols], pm2[:, :cols],
                                                    vcol(0, l, 1), None, op0=OP.add)
                            realc = min(512, max(0, n_sub - gg * 512))
                            if realc > 0:
                                nc.vector.bn_stats(stats[:, gg * 6:(gg + 1) * 6],
                                                   at[:, co:co + realc])
                        g += nblk

                op_i = 0
                ps_of = {}
                for (bb, c0, k, h0end) in calls_meta:
                    xt = xsp.tile([128, cfg["CALLCH"], H], BF16, tag="x")
                    n = k * 128
                    nc.gpsimd.dma_gather(
                        xt[:, :k, :], rep_sub[bb * S_sub:(bb + 1) * S_sub, :],
                        qidx_sb[:, (c0 * 128) // 16:(c0 * 128 + n) // 16], n, n, H,
                        single_packet=False, queue_num=nextq())
                    # S = relu(1 - (iota - d)^2) on the Scalar engine, in
                    # quads: per-chunk Square (per-chunk bias), one fused Relu
                    # over 4 chunks (constant bias amortizes ACT overhead).
                    St4_of = {}
                    for jq in range(0, k, 4):
                        kq = min(4, k - jq)
                        sq4 = sgp.tile([128, 4 * GRPW], BF16, tag="sgq")
                        for t in range(kq):
                            nc.scalar.activation(
                                sq4[:, t * GRPW:(t + 1) * GRPW], iota_sb[:],
                                AF.Square,
                                bias=drel_sb[:, op_i + jq + t:op_i + jq + t + 1],
                                scale=1.0)
                        St4 = sgp.tile([128, 4 * GRPW], BF16, tag="sgen")
                        nc.scalar.activation(St4[:, :kq * GRPW], sq4[:, :kq * GRPW],
                                             AF.Relu, bias=1.0, scale=-1.0)
                        for t in range(kq):
                            St4_of[jq + t] = (St4, t * GRPW)
                    for j in range(k):
                        ops = chunks_meta[c0 + j]
                        for (g, first, stop, gfirst) in ops:
                            St4, sco = St4_of[j]
                            if first:
                                ps_of[g] = psA.tile([128, GRPW], F32, tag="agg2",
                                                    name=f"agg{g % 2}")
                            cols = min(GRPW, S_sub - g * GRPW)
                            nc.tensor.matmul(ps_of[g][:, :cols], xt[:, j, :],
                                             St4[:, sco:sco + cols],
                                             start=first, stop=stop)
                            if stop:
                                pt = ps_of.pop(g)
                                at, co = acc_at(g)
                                if gfirst:
                                    nc.vector.tensor_copy(
                                        at[:, co:co + cols], pt[:, :cols])
                                else:
                                    nc.vector.tensor_tensor(
                                        at[:, co:co + cols],
                                        at[:, co:co + cols],
                                        pt[:, :cols], op=OP.add)
                            op_i += 1
                    if h0end:
                        epilogue(0, SPLITC // 512)
                epilogue(SPLITC // 512, (S_sub + 511) // 512)
                bn_tail(0, l, acc512, stats, S_sub, n_sub, hown_sub, S_sub // 128)

            # ---- glob phase GIN layer ----
            def gin_glob(l):
                Sp, n_real, rep, hown = S_glob, n_glob, rep_glob, hown_glob
                n_tiles = Sp // 128
                nc.gpsimd.collective_compute(
                    "AllGather", OP.bypass, RG, [hown.ap().opt()], [rep.ap().opt()])
                z2 = resp.tile([128, S_glob], BF16, tag="z2g")
                stats = smp.tile([128, 64 * 6], F32, tag="stats")
                chunks = []
                for t in range(n_tiles):
                    for j in range(int(CtG[t])):
                        chunks.append((t, j == 0, j == int(CtG[t]) - 1))
                psum_of = {}
                groups = [chunks[i:i + GRP] for i in range(0, len(chunks), GRP)]
                ch_base = 0
                zgrp_tiles = {}

                def run_mlp(g):
                    zg = zgrp_tiles.pop(g)
                    cols = min(512, Sp - g * 512)
                    pm = psM.tile([128, 512], F32, tag="m1")
                    nc.tensor.matmul(pm[:, :cols], W1g[:, l * H:(l + 1) * H], zg[:, :cols],
                                     start=True, stop=True)
                    z1 = zzp.tile([128, 512], BF16, tag="z1")
                    nc.scalar.activation(z1[:, :cols], pm[:, :cols], AF.Relu,
                                         bias=vcol(1, l, 0), scale=1.0)
                    pm2 = psM.tile([128, 512], F32, tag="m2")
                    nc.tensor.matmul(pm2[:, :cols], W2g[:, l * H:(l + 1) * H], z1[:, :cols],
                                     start=True, stop=True)
                    nc.vector.tensor_scalar(z2[:, g * 512:g * 512 + cols], pm2[:, :cols],
                                            vcol(1, l, 1), None, op0=OP.add)
                    realc = min(512, max(0, n_real - g * 512))
                    if realc > 0:
                        nc.vector.bn_stats(stats[:, g * 6:(g + 1) * 6],
                                           z2[:, g * 512:g * 512 + realc])

                for gci, grp in enumerate(groups):
                    nch = len(grp)
                    xt = xsp.tile([128, cfg["CALLCH"], H], BF16, tag="x")
                    n = nch * 128
                    nc.gpsimd.dma_gather(
                        xt[:, :nch, :], rep[:, :],
                        gg_sb[:, ch_base * 8:(ch_base + nch) * 8],
                        n, n, H, single_packet=False, queue_num=nextq())
                    st = xsp.tile([128, GRP * 128], BF16, tag="s")
                    nc.sync.dma_start(st[:, :n], t_Sglob[:, ch_base * 128:ch_base * 128 + n])
                    for j, (t, first, lastc) in enumerate(grp):
                        if first:
                            psum_of[t] = psA.tile([128, 512], F32, tag="agg",
                                                  name=f"aggg{t % 8}")
                        nc.tensor.matmul(psum_of[t][:, :128], xt[:, j, :],
                                         st[:, j * 128:(j + 1) * 128],
                                         start=first, stop=lastc)
                        if lastc:
                            g = (t * 128) // 512
                            if g not in zgrp_tiles:
                                zgrp_tiles[g] = zzp.tile([128, 512], BF16, tag="zg",
                                                         name=f"zg{g % 4}")
                                hgt = zzp.tile([128, 512], BF16, tag="hg")
                                cols = min(512, Sp - g * 512)
                                nc.sync.dma_start(hgt[:, :cols],
                                                  hown[g * 512:g * 512 + cols, :],
                                                  transpose=True)
                                zgrp_tiles[(g, "h")] = hgt
                            hgt = zgrp_tiles[(g, "h")]
                            cc = t * 128 - g * 512
                            pt = psum_of.pop(t)
                            nc.vector.scalar_tensor_tensor(
                                zgrp_tiles[g][:, cc:cc + 128], hgt[:, cc:cc + 128],
                                vcol(1, l, 4), pt[:, :128],
                                op0=OP.mult, op1=OP.add)
                            if (t * 128 + 128) % 512 == 0 or t == n_tiles - 1:
                                zgrp_tiles.pop((g, "h"))
                                run_mlp(g)
                    ch_base += nch
                bn_tail(1, l, z2, stats, S_glob, n_glob, hown_glob, n_tiles)

            # ---- sub phase ----
            for l in range(L):
                gin_sub(l)

            # ---- phase boundary: weighted mean of 2 roots -> hown_glob ----
            r0_sb = idxp.tile([128, S_glob // 16], I16, tag="r0")
            r4_sb = idxp.tile([128, S_glob // 16], I16, tag="r4")
            nc.sync.dma_start(r0_sb[:], t_r0[:])
            nc.sync.dma_start(r4_sb[:], t_r4[:])
            r0b = segp.tile([128, S_glob // 128, H], BF16, tag="seg")
            r4b = segp.tile([128, S_glob // 128, H], BF16, tag="seg")
            nc.gpsimd.dma_gather(r0b[:], hown_sub[:], r0_sb[:], S_glob, S_glob, H,
                                 single_packet=False, queue_num=nextq())
            nc.gpsimd.dma_gather(r4b[:], hown_sub[:], r4_sb[:], S_glob, S_glob, H,
                                 single_packet=False, queue_num=nextq())
            inv_temp = 1.0 / float(cfg["TEMP"])
            for t in range(S_glob // 128):
                lpt = smp.tile([128, 2], F32, tag="lpt")
                nc.sync.dma_start(lpt[:], t_lp[t * 128:(t + 1) * 128, :])
                d = smp.tile([128, 2], F32, tag="d")
                nc.vector.tensor_tensor(d[:, 0:1], lpt[:, 0:1], lpt[:, 1:2], op=OP.subtract)
                nc.scalar.activation(d[:, 0:1], d[:, 0:1], AF.Sigmoid, bias=0.0,
                                     scale=inv_temp)
                nc.vector.tensor_scalar(d[:, 1:2], d[:, 0:1], -1.0, 1.0,
                                        op0=OP.mult, op1=OP.add)
                hb = segp.tile([128, H], BF16, tag="hb")
                nc.vector.tensor_scalar(hb[:], r0b[:, t, :], d[:, 0:1], None, op0=OP.mult)
                nc.vector.scalar_tensor_tensor(hb[:], r4b[:, t, :], d[:, 1:2], hb[:],
                                               op0=OP.mult, op1=OP.add)
                nc.sync.dma_start(hown_glob[t * 128:(t + 1) * 128, :], hb[:])

            # ---- glob phase ----
            for l in range(L):
                gin_glob(l)

            # ---- readout ----
            prd = psM.tile([128, G], F32, tag="m1")
            for t in range(S_glob // 128):
                hrow = segp.tile([128, H], BF16, tag="hrow")
                nc.sync.dma_start(hrow[:], hown_glob[t * 128:(t + 1) * 128, :])
                srdt = segp.tile([128, G], BF16, tag="srdt")
                nc.sync.dma_start(srdt[:], t_Srd[:, t * G:(t + 1) * G])
                nc.tensor.matmul(prd[:], hrow[:], srdt[:],
                                 start=(t == 0), stop=(t == S_glob // 128 - 1))
            rd_sb = constp.tile([128, G], F32, tag="rdsb")
            nc.vector.tensor_copy(rd_sb[:], prd[:])
            nc.sync.dma_start(rd_in[:], rd_sb[:])
            nc.gpsimd.collective_compute(
                "AllReduce", OP.add, RG, [rd_in.ap().opt()], [rd_out.ap().opt()])
            rd2 = constp.tile([128, G], F32, tag="rd2")
            nc.sync.dma_start(rd2[:], rd_out[:])
            idf = constp.tile([128, 128], F32, tag="idf")
            nc.sync.dma_start(idf[:], t_idn_f[:])
            for g in range((G + 127) // 128):
                cols = min(128, G - g * 128)
                pt = psA.tile([128, 512], F32, tag="agg")
                nc.tensor.transpose(pt[:cols, :128], rd2[:, g * 128:g * 128 + cols], idf[:])
                ot = constp.tile([128, 128], F32, tag="ot")
                nc.vector.tensor_copy(ot[:cols, :], pt[:cols, :128])
                nc.sync.dma_start(t_out[g * 128:g * 128 + cols, :], ot[:cols, :])

    nc.compile()
    return nc


def build_inmaps(plan, weights):
    cfg = plan["cfg"]
    W, H, L = cfg["W"], cfg["H"], cfg["L"]
    n_glob, S_glob = cfg["n_glob"], cfg["S_glob"]
    lp = np.asarray(weights["log_probs"], np.float32)
    maps = []
    vecs = np.zeros((128, 10 * L), np.float32)
    for ph, pre in ((0, "sub"), (1, "glob")):
        for l in range(L):
            base = ph * 5 * L + l * 5
            vecs[:, base + 0] = np.asarray(weights[f"{pre}_b1"][l], np.float32)
            vecs[:, base + 1] = np.asarray(weights[f"{pre}_b2"][l], np.float32)
            vecs[:, base + 2] = np.asarray(weights[f"{pre}_gamma"][l], np.float32)
            vecs[:, base + 3] = np.asarray(weights[f"{pre}_beta"][l], np.float32)
            vecs[:, base + 4] = 1.0 + np.float32(weights[f"{pre}_eps"][l])
    idn = np.eye(128)
    iota = np.tile(np.arange(cfg["GRPW"], dtype=np.float32), (128, 1)).astype(F16)
    common = {
        "atom": np.asarray(weights["atom_table"], np.float32).astype(BF),
        "W1s": np.asarray(weights["sub_W1"], np.float32).astype(BF),
        "W2s": np.asarray(weights["sub_W2"], np.float32).astype(BF),
        "W1g": np.asarray(weights["glob_W1"], np.float32).astype(BF),
        "W2g": np.asarray(weights["glob_W2"], np.float32).astype(BF),
        "vecs": vecs,
        "idnbf": idn.astype(BF),
        "idnf": idn.astype(np.float32),
        "iota16": iota,
    }
    atom_bf = np.asarray(weights["atom_table"], np.float32).astype(BF)
    n_sub, S_sub = cfg["n_sub"], cfg["S_sub"]
    h0_all = atom_bf[plan["aid_ids"]]
    for c in range(W):
        h0c = np.zeros((S_sub, H), BF)
        h0c[:n_sub] = h0_all[c * n_sub:(c + 1) * n_sub]
        lpc = np.zeros((S_glob, 2), np.float32)
        lpc[:n_glob] = lp[plan["lp_sel"][c]]
        m = dict(common)
        m.update({
            "qidx": plan["sub_qidx"][c],
            "drel": plan["sub_drel"][c],
            "ggidx": plan["glob_g2_idx"][c],
            "h0": h0c,
            "r0idx": plan["r0"][c],
            "r4idx": plan["r4"][c],
            "Sglob": plan["glob_S"][c],
            "Srd": plan["Sg"][c],
            "lp": lpc,
        })
        maps.append(m)
    return maps


def kernel(**inputs):
    import numpy as np
    cfg = dict(DEF_CFG)
    inp = {k: np.asarray(v) for k, v in inputs.items()}
    plan = build_plan(cfg, inp["x"], inp["edge_index"], inp["sub_node_map"],
                      inp["sub_edge_index"], inp["root_idx"], inp["target_batch"],
                      inp["batch"])
    nc = build_graph(plan)
    maps = build_inmaps(plan, inp)
    from concourse import bass_utils
    res = bass_utils.run_bass_kernel_spmd(nc, maps, core_ids=list(range(cfg["W"])),
                                          trace=False)
    return np.asarray(res.results[0]["out"], np.float32)



# revision 2
# speedup vs baseline: 1.0004x; 1.0004x over previous
"""Trainium2 Bass kernel: distributed GIN graph encoder on 8 NeuronCores.

v5: layer-0 replica precomputed on host (no initial AllGather); transposed
h kept resident in SBUF for both phases (no per-layer transposed DMA
reloads in epilogue/BN); S one-hot generation split between Scalar
(relu(1-(iota-d)^2)) and Vector (is_equal) engines to balance load.
Sub-phase aggregation keeps the (bucket x 256-dst-group) single-stage
gather with SBUF bf16 accumulator.
"""

import numpy as np
import ml_dtypes

BF = ml_dtypes.bfloat16
F16 = np.float16

DEF_CFG = dict(
    W=8, H=128, L=4,
    n_sub=30000, S_sub=30080,     # per-core real/padded sub rows
    n_glob=3750, S_glob=3840,
    VA=128, G=300, TEMP=0.5, BN_EPS=1e-5,
    GRPW=256,                     # dst psum group width (sub)
    CALLCH=28,                    # chunks per gather call (sub)
    GRP=16,                       # glob: chunks per gather call
    QACT=3, QCYC=4,               # S-gen quads: QACT of QCYC on Scalar, rest DVE
)


def _pack16(arr):
    """idx array (n,) int -> [128, n/16] int16 tile content (pos i -> [i%16, i//16])."""
    a = np.asarray(arr, np.int16)
    assert len(a) % 16 == 0
    t = a.reshape(-1, 16).T
    return np.tile(t, (8, 1))


def _pad128(n):
    return (n + 127) // 128 * 128


def build_plan(cfg, x, edge_index, sub_node_map, sub_edge_index, root_idx,
               target_batch, batch):
    W, H = cfg["W"], cfg["H"]
    n_sub, S_sub = cfg["n_sub"], cfg["S_sub"]
    n_glob, S_glob = cfg["n_glob"], cfg["S_glob"]
    GRPW = cfg["GRPW"]
    plan = {"cfg": cfg}
    N = n_glob * W

    # ---------- sub phase edge plan (single-stage, shared budgets) ----------
    src, dst = np.asarray(sub_edge_index[0]), np.asarray(sub_edge_index[1])
    owner = dst // n_sub
    dst_local = dst % n_sub
    src_row = (src // n_sub) * S_sub + (src % n_sub)   # row in padded replica
    ngrp = (S_sub + GRPW - 1) // GRPW

    # group edges per (core, bucket, dst-group); shared budget = max over cores
    per_core = []
    cnt = np.zeros((W, W, ngrp), np.int64)   # [core, bucket, group]
    for c in range(W):
        m = owner == c
        sc, dl = src_row[m], dst_local[m]
        b = sc // S_sub
        q = sc % S_sub
        g = dl // GRPW
        order = np.lexsort((q, g, b))
        b, q, dl, g = b[order], q[order], dl[order], g[order]
        per_core.append((b, q, dl, g))
        np.add.at(cnt[c], (b, g), 1)
    budget = _pad128(cnt.max(axis=0))        # [bucket, group] slots (multiple of 128)
    budget = np.maximum(budget, 128)
    GH = (ngrp + 1) // 2
    if (GH * GRPW) % 512:
        GH += 1          # keep the acc0/acc1 split 512-aligned for MLP groups
    nslots = int(budget.sum())
    nchunks = int((budget // 128).sum())
    # shared chunk list + calls, ordered (half, bucket, group)
    cell_off2 = np.zeros((W, ngrp), np.int64)
    off = 0
    chunks = []   # per chunk: [(g, first, stop, gfirst)]
    calls = []    # (bucket, chunk_start, nchunks, end_of_half0)
    for hh in range(2):
        glo, ghi = hh * GH, min((hh + 1) * GH, ngrp)
        for bb in range(W):
            c0 = len(chunks)
            for g in range(glo, ghi):
                cell_off2[bb, g] = off
                off += int(budget[bb, g])
                nch = int(budget[bb, g]) // 128
                for j in range(nch):
                    chunks.append([(g, j == 0, j == nch - 1, bb == 0)])
            done = 0
            nb = len(chunks) - c0
            while done < nb:
                k = min(cfg["CALLCH"], nb - done)
                done += k
                calls.append((bb, c0 + done - k, k,
                              hh == 0 and bb == W - 1 and done >= nb))
    assert off == nslots
    # per-core slot data; drel stores NEGATED dst_rel (ACT Square bias), pad -> +1
    sub_qidx, sub_drel = [], []
    for c in range(W):
        b, q, dl, g = per_core[c]
        qpad = np.zeros(nslots, np.int64)
        dpad = np.full(nslots, -1, np.int64)
        cell = b * ngrp + g
        uniq, start_idx, counts = np.unique(cell, return_index=True, return_counts=True)
        for u, s0, k in zip(uniq, start_idx, counts):
            sl = cell_off2[u // ngrp, u % ngrp] + np.arange(k)
            qpad[sl] = q[s0:s0 + k]
            dpad[sl] = dl[s0:s0 + k] - (u % ngrp) * GRPW
        sub_qidx.append(_pack16(qpad))
        sub_drel.append((-dpad.reshape(-1, 128).T).astype(np.float32))  # [128, nchunks]
    plan["sub_nslots"] = nslots
    plan["sub_nops"] = nchunks
    plan["sub_qidx"] = sub_qidx
    plan["sub_drel"] = sub_drel
    plan["sub_chunks"] = chunks
    plan["sub_calls"] = calls
    plan["sub_ngrp"] = ngrp
    plan["sub_GH"] = GH

    # ---------- glob phase edge plan ----------
    gsrc, gdst = np.asarray(edge_index[0]), np.asarray(edge_index[1])
    gowner = gdst // n_glob
    gdst_local = gdst % n_glob
    gsrc_row = (gsrc // n_glob) * S_glob + (gsrc % n_glob)
    n_tiles_glob = S_glob // 128
    per_core_g = []
    CtG = np.zeros(n_tiles_glob, np.int64)
    for c in range(W):
        m = gowner == c
        sc, dl = gsrc_row[m], gdst_local[m]
        order = np.argsort(dl, kind="stable")
        sc, dl = sc[order], dl[order]
        per_core_g.append((sc, dl))
        gcnt = np.bincount(dl // 128, minlength=n_tiles_glob)
        CtG = np.maximum(CtG, (gcnt + 127) // 128)
    CtG = np.maximum(CtG, 1)
    plan["glob_Ct"] = CtG
    TOTCHG = int(CtG.sum())
    plan["glob_TOTCH"] = TOTCHG
    tile_slot_off_g = np.zeros(n_tiles_glob + 1, np.int64)
    tile_slot_off_g[1:] = np.cumsum(CtG * 128)
    gg_idx_cores, Sg_cores = [], []
    for c in range(W):
        sc, dl = per_core_g[c]
        g2 = np.zeros(TOTCHG * 128, np.int64)
        S = np.zeros((TOTCHG * 128, 128), BF)
        tl = dl // 128
        for t in range(n_tiles_glob):
            mt = tl == t
            k = int(mt.sum())
            slots = tile_slot_off_g[t] + np.arange(k)
            g2[slots] = sc[mt]
            S[slots, dl[mt] % 128] = BF(1.0)
        gg_idx_cores.append(_pack16(g2))
        Sg_cores.append(S)
    plan["glob_g2_idx"] = gg_idx_cores
    plan["glob_S"] = [S.reshape(TOTCHG, 128, 128).transpose(1, 0, 2).reshape(128, TOTCHG * 128)
                      for S in Sg_cores]

    # ---------- atom encode (host) ----------
    plan["aid_ids"] = np.asarray(x)[np.asarray(sub_node_map)]

    # ---------- phase boundary (roots) ----------
    tb_arr = np.asarray(target_batch)
    ri = np.asarray(root_idx)
    order = np.argsort(tb_arr, kind="stable")
    assert (np.bincount(tb_arr, minlength=N) == 2).all(), "need exactly 2 roots/node"
    r_sorted = ri[order].reshape(N, 2)
    lp_order = order.reshape(N, 2)
    plan["r0"], plan["r4"], plan["lp_sel"] = [], [], []
    for c in range(W):
        r = r_sorted[c * n_glob:(c + 1) * n_glob]
        lo = c * n_sub
        assert ((r >= lo) & (r < lo + n_sub)).all(), "roots must be core-local"
        r0 = np.zeros(S_glob, np.int64)
        r4 = np.zeros(S_glob, np.int64)
        r0[:n_glob] = r[:, 0] - lo
        r4[:n_glob] = r[:, 1] - lo
        plan["r0"].append(_pack16(r0))
        plan["r4"].append(_pack16(r4))
        plan["lp_sel"].append(lp_order[c * n_glob:(c + 1) * n_glob])

    # ---------- readout ----------
    b_arr = np.asarray(batch)
    plan["Sg"] = []
    for c in range(W):
        Srd = np.zeros((S_glob, cfg["G"]), BF)
        ids = b_arr[c * n_glob:(c + 1) * n_glob]
        Srd[np.arange(n_glob), ids] = BF(1.0)
        nt = S_glob // 128
        plan["Sg"].append(Srd.reshape(nt, 128, cfg["G"]).transpose(1, 0, 2).reshape(128, nt * cfg["G"]))
    return plan


def _install_queue_aware_lanes():
    """Make Tile's DMASW lane assignment queue-aware: lane = queue*2 + rr."""
    import concourse.tile_sem_assignment as tsa
    if getattr(tsa, "_qaware_installed", False):
        return
    orig = tsa.TileClockTick._assign_tick
    import concourse.mybir as mb

    def patched(self, inst):
        qn = getattr(inst, "queue_num", None)
        if (qn is not None and inst.engine == mb.EngineType.Pool
                and isinstance(inst, tsa.DMAInst)
                and self.swdge_sem_count == 8):
            rr_map = getattr(self, "_q_rr", None)
            if rr_map is None:
                rr_map = self._q_rr = {}
            sub = rr_map.get(qn, 0)
            rr_map[qn] = (sub + 1) % 2
            lane = qn * 2 + sub
            save = self.next_sw_dma_idx
            self.next_sw_dma_idx = lane
            try:
                return orig(self, inst)
            finally:
                self.next_sw_dma_idx = save
        return orig(self, inst)

    tsa.TileClockTick._assign_tick = patched
    tsa._qaware_installed = True


def build_graph(plan):
    from concourse import bass, mybir, bacc
    import concourse.tile as tile

    cfg = plan["cfg"]
    W, H, L = cfg["W"], cfg["H"], cfg["L"]
    n_sub, S_sub = cfg["n_sub"], cfg["S_sub"]
    n_glob, S_glob = cfg["n_glob"], cfg["S_glob"]
    G = cfg["G"]
    GRPW = cfg["GRPW"]
    BF16 = mybir.dt.bfloat16
    FP16 = mybir.dt.float16
    F32 = mybir.dt.float32
    I16 = mybir.dt.int16
    AF = mybir.ActivationFunctionType
    OP = mybir.AluOpType
    GRP = cfg["GRP"]
    ngrp = plan["sub_ngrp"]
    QACT, QCYC = cfg["QACT"], cfg["QCYC"]

    _install_queue_aware_lanes()
    nc = bacc.Bacc("TRN2", target_bir_lowering=False, debug=False, num_devices=W,
                   num_swdge_queues=4)

    def inp(name, shape, dt):
        return nc.dram_tensor(name, shape, dt, kind="ExternalInput")

    TOTCHG = plan["glob_TOTCH"]
    nslots, nops = plan["sub_nslots"], plan["sub_nops"]
    t_qidx = inp("qidx", [128, nslots // 16], I16)
    t_drel = inp("drel", [128, nops], F32)
    t_iota = inp("iota16", [128, GRPW], FP16)
    t_iotan = inp("iotan", [128, GRPW], BF16)
    t_gg = inp("ggidx", [128, TOTCHG * 8], I16)
    t_rep0 = inp("rep0", [W * S_sub, H], BF16)
    t_h0T = inp("h0T", [128, S_sub], BF16)
    t_r0 = inp("r0idx", [128, S_glob // 16], I16)
    t_r4 = inp("r4idx", [128, S_glob // 16], I16)
    t_Sglob = inp("Sglob", [128, TOTCHG * 128], BF16)
    t_Srd = inp("Srd", [128, (S_glob // 128) * G], BF16)
    t_idn_bf = inp("idnbf", [128, 128], BF16)
    t_idn_f = inp("idnf", [128, 128], F32)
    t_W1s = inp("W1s", [L, H, H], BF16)
    t_W2s = inp("W2s", [L, H, H], BF16)
    t_W1g = inp("W1g", [L, H, H], BF16)
    t_W2g = inp("W2g", [L, H, H], BF16)
    t_vecs = inp("vecs", [128, 10 * L], F32)
    t_lp = inp("lp", [S_glob, 2], F32)
    t_out = nc.dram_tensor("out", [G, H], F32, kind="ExternalOutput")

    rep_sub = nc.dram_tensor("rep_sub", [W * S_sub, H], BF16, addr_space="Shared")
    rep_glob = nc.dram_tensor("rep_glob", [W * S_glob, H], BF16, addr_space="Shared")
    hown_sub = nc.dram_tensor("hown_sub", [S_sub, H], BF16)
    hown_glob = nc.dram_tensor("hown_glob", [S_glob, H], BF16)
    ar_in = nc.dram_tensor("ar_in", [128, 2], F32)
    ar_out = nc.dram_tensor("ar_out", [128, 2], F32, addr_space="Shared")
    rd_in = nc.dram_tensor("rd_in", [128, G], F32)
    rd_out = nc.dram_tensor("rd_out", [128, G], F32, addr_space="Shared")

    RG = [list(range(W))]
    _qrr = [0]

    def nextq():
        q = (_qrr[0] % 8) // 2
        _qrr[0] += 1
        return q

    CtG = plan["glob_Ct"]
    chunks_meta = plan["sub_chunks"]
    calls_meta = plan["sub_calls"]
    GH = plan["sub_GH"]
    SPLITC = GH * GRPW

    with tile.TileContext(nc) as tc:
        with (
            tc.tile_pool(name="const", bufs=1) as constp,
            tc.tile_pool(name="xs", bufs=3) as xsp,
            tc.tile_pool(name="sgen", bufs=2) as sgp,
            tc.tile_pool(name="zz", bufs=3) as zzp,
            tc.tile_pool(name="small", bufs=2) as smp,
            tc.tile_pool(name="stg", bufs=2) as stgp,
            tc.tile_pool(name="psA", bufs=2, space="PSUM") as psA,
            tc.tile_pool(name="psM", bufs=1, space="PSUM") as psM,
            tc.tile_pool(name="psT", bufs=2, space="PSUM") as psT,
        ):
            # ---- constants resident ----
            vecs = constp.tile([128, 10 * L], F32)
            nc.sync.dma_start(vecs[:], t_vecs[:])
            W1s = constp.tile([128, L * H], BF16)
            W2s = constp.tile([128, L * H], BF16)
            W1g = constp.tile([128, L * H], BF16)
            W2g = constp.tile([128, L * H], BF16)
            for l in range(L):
                nc.sync.dma_start(W1s[:, l * H:(l + 1) * H], t_W1s[l])
                nc.sync.dma_start(W2s[:, l * H:(l + 1) * H], t_W2s[l])
                nc.sync.dma_start(W1g[:, l * H:(l + 1) * H], t_W1g[l])
                nc.sync.dma_start(W2g[:, l * H:(l + 1) * H], t_W2g[l])
            idn = constp.tile([128, 128], BF16, tag="idn")
            nc.sync.dma_start(idn[:], t_idn_bf[:])
            qidx_sb = constp.tile([128, nslots // 16], I16, tag="qidx")
            nc.sync.dma_start(qidx_sb[:], t_qidx[:])
            drel_sb = constp.tile([128, nops], F32, tag="drel")
            nc.sync.dma_start(drel_sb[:], t_drel[:])
            iota_sb = constp.tile([128, GRPW], FP16, tag="iota")
            nc.sync.dma_start(iota_sb[:], t_iota[:])
            iotan_sb = constp.tile([128, GRPW], BF16, tag="iotan")
            nc.sync.dma_start(iotan_sb[:], t_iotan[:])

            def vcol(phase, l, j):
                return vecs[:, (phase * 5 * L + l * 5 + j):(phase * 5 * L + l * 5 + j) + 1]

            # shared BN scalar computation (stats -> mu tile) ---------------
            def bn_head(phase, l, stats, n_real):
                ngr = (n_real + 511) // 512
                mv = smp.tile([128, 2], F32, tag="mv")
                nc.vector.bn_aggr(mv[:], stats[:, :ngr * 6])
                sin = smp.tile([128, 2], F32, tag="sin")
                nc.vector.tensor_tensor(sin[:, 1:2], mv[:, 0:1], mv[:, 0:1], op=OP.mult)
                nc.vector.tensor_tensor(sin[:, 1:2], sin[:, 1:2], mv[:, 1:2], op=OP.add)
                nc.vector.tensor_copy(sin[:, 0:1], mv[:, 0:1])
                nc.sync.dma_start(ar_in[:], sin[:])
                nc.gpsimd.collective_compute(
                    "AllReduce", OP.add, RG, [ar_in.ap().opt()], [ar_out.ap().opt()])
                sg = smp.tile([128, 2], F32, tag="sg")
                nc.sync.dma_start(sg[:], ar_out[:])
                mu = smp.tile([128, 4], F32, tag="mu")
                nc.scalar.mul(mu[:, 0:1], sg[:, 0:1], 1.0 / W)
                nc.scalar.mul(mu[:, 1:2], sg[:, 1:2], 1.0 / W)
                nc.vector.tensor_tensor(mu[:, 2:3], mu[:, 0:1], mu[:, 0:1], op=OP.mult)
                nc.vector.tensor_tensor(mu[:, 1:2], mu[:, 1:2], mu[:, 2:3], op=OP.subtract)
                nc.vector.tensor_scalar(mu[:, 1:2], mu[:, 1:2], float(cfg["BN_EPS"]), None,
                                        op0=OP.add)
                nc.scalar.activation(mu[:, 1:2], mu[:, 1:2], AF.Sqrt, bias=0.0, scale=1.0)
                nc.vector.reciprocal(mu[:, 1:2], mu[:, 1:2])
                nc.vector.tensor_tensor(mu[:, 2:3], vcol(phase, l, 2), mu[:, 1:2], op=OP.mult)
                nc.vector.tensor_tensor(mu[:, 3:4], mu[:, 0:1], mu[:, 2:3], op=OP.mult)
                nc.vector.tensor_tensor(mu[:, 3:4], vcol(phase, l, 3), mu[:, 3:4], op=OP.subtract)
                return mu

            # normalize z (SBUF, h^T layout) + residual into hT, write hown
            def bn_apply(mu, zread, hT, hown, Sp, n_tiles):
                stgt = stgp.tile([128, 16, 128], BF16, tag="stg")
                stg_fill = 0
                stg_t0 = 0
                for g in range((Sp + 511) // 512):
                    base = g * 512
                    cols = min(512, Sp - base)
                    zt_, co_ = zread(g)
                    hn = zzp.tile([128, 512], BF16, tag="hn")
                    nc.vector.tensor_scalar(hn[:, :cols], zt_[:, co_:co_ + cols],
                                            mu[:, 2:3], mu[:, 3:4],
                                            op0=OP.mult, op1=OP.add)
                    nc.vector.tensor_tensor(hT[:, base:base + cols], hn[:, :cols],
                                            hT[:, base:base + cols], op=OP.add)
                    for q in range(cols // 128):
                        t = base // 128 + q
                        pt = psT.tile([128, 128], BF16, tag="tr")
                        nc.tensor.transpose(pt[:], hT[:, t * 128:(t + 1) * 128], idn[:])
                        nc.vector.tensor_copy(stgt[:, stg_fill, :], pt[:])
                        stg_fill += 1
                        if stg_fill == 16 or t == n_tiles - 1:
                            nc.sync.dma_start(
                                hown.ap().rearrange("(c p) h -> p c h", p=128)[:, stg_t0:stg_t0 + stg_fill, :],
                                stgt[:, :stg_fill, :])
                            stg_t0 += stg_fill
                            stg_fill = 0
                            if t != n_tiles - 1:
                                stgt = stgp.tile([128, 16, 128], BF16, tag="stg")

            # ================= sub phase =================
            with tc.tile_pool(name="subres", bufs=1) as subres:
                hTs = subres.tile([128, S_sub], BF16, tag="hTs")
                nc.sync.dma_start(hTs[:], t_h0T[:])

                def gin_sub(l):
                    if l > 0:
                        nc.gpsimd.collective_compute(
                            "AllGather", OP.bypass, RG,
                            [hown_sub.ap().opt()], [rep_sub.ap().opt()])
                        rep_src = rep_sub
                    else:
                        rep_src = t_rep0
                    acc0 = subres.tile([128, SPLITC], BF16, tag="acc0")
                    acc1 = subres.tile([128, S_sub - SPLITC], BF16, tag="acc1")
                    stats = smp.tile([128, 64 * 6], F32, tag="stats")

                    def acc_at(g):
                        if g < GH:
                            return acc0, g * GRPW
                        return acc1, g * GRPW - SPLITC

                    def acc512(g):
                        co = g * 512
                        if co < SPLITC:
                            return acc0, co
                        return acc1, co - SPLITC

                    def epilogue(g0, g1):
                        for g in range(g0, g1):
                            cols = min(512, S_sub - g * 512)
                            at, co = acc512(g)
                            zg = zzp.tile([128, 512], BF16, tag="zg2")
                            nc.vector.scalar_tensor_tensor(
                                zg[:, :cols], hTs[:, g * 512:g * 512 + cols],
                                vcol(0, l, 4), at[:, co:co + cols],
                                op0=OP.mult, op1=OP.add)
                            pm = psM.tile([128, 512], F32, tag="m1")
                            nc.tensor.matmul(pm[:, :cols], W1s[:, l * H:(l + 1) * H],
                                             zg[:, :cols], start=True, stop=True)
                            z1 = zzp.tile([128, 512], BF16, tag="z1")
                            nc.scalar.activation(z1[:, :cols], pm[:, :cols], AF.Relu,
                                                 bias=vcol(0, l, 0), scale=1.0)
                            pm2 = psM.tile([128, 512], F32, tag="m2")
                            nc.tensor.matmul(pm2[:, :cols], W2s[:, l * H:(l + 1) * H],
                                             z1[:, :cols], start=True, stop=True)
                            nc.vector.tensor_scalar(at[:, co:co + cols], pm2[:, :cols],
                                                    vcol(0, l, 1), None, op0=OP.add)
                            realc = min(512, max(0, n_sub - g * 512))
                            if realc > 0:
                                nc.vector.bn_stats(stats[:, g * 6:(g + 1) * 6],
                                                   at[:, co:co + realc])

                    op_i = 0
                    ps_of = {}
                    for (bb, c0, k, h0end) in calls_meta:
                        xt = xsp.tile([128, cfg["CALLCH"], H], BF16, tag="x")
                        n = k * 128
                        nc.gpsimd.dma_gather(
                            xt[:, :k, :], rep_src[bb * S_sub:(bb + 1) * S_sub, :],
                            qidx_sb[:, (c0 * 128) // 16:(c0 * 128 + n) // 16], n, n, H,
                            single_packet=False, queue_num=nextq())
                        # S-gen: quads on Scalar (Square+fused Relu), rest on
                        # DVE (is_equal against negated iota).
                        St4_of = {}
                        for jq in range(0, k, 4):
                            kq = min(4, k - jq)
                            if (jq // 4) % QCYC < QACT:
                                sq4 = sgp.tile([128, 4 * GRPW], BF16, tag="sgq")
                                for t in range(kq):
                                    nc.scalar.activation(
                                        sq4[:, t * GRPW:(t + 1) * GRPW], iota_sb[:],
                                        AF.Square,
                                        bias=drel_sb[:, op_i + jq + t:op_i + jq + t + 1],
                                        scale=1.0)
                                St4 = sgp.tile([128, 4 * GRPW], BF16, tag="sgen")
                                nc.scalar.activation(St4[:, :kq * GRPW], sq4[:, :kq * GRPW],
                                                     AF.Relu, bias=1.0, scale=-1.0)
                                for t in range(kq):
                                    St4_of[jq + t] = (St4, t * GRPW)
                            else:
                                for t in range(kq):
                                    sd = sgp.tile([128, GRPW], BF16, tag="sgd", bufs=6)
                                    nc.vector.tensor_scalar(
                                        sd[:], iotan_sb[:],
                                        drel_sb[:, op_i + jq + t:op_i + jq + t + 1],
                                        None, op0=OP.is_equal)
                                    St4_of[jq + t] = (sd, 0)
                        for j in range(k):
                            ops = chunks_meta[c0 + j]
                            for (g, first, stop, gfirst) in ops:
                                St4, sco = St4_of[j]
                                if first:
                                    ps_of[g] = psA.tile([128, GRPW], F32, tag="agg2",
                                                        name=f"agg{g % 2}")
                                cols = min(GRPW, S_sub - g * GRPW)
                                nc.tensor.matmul(ps_of[g][:, :cols], xt[:, j, :],
                                                 St4[:, sco:sco + cols],
                                                 start=first, stop=stop)
                                if stop:
                                    pt = ps_of.pop(g)
                                    at, co = acc_at(g)
                                    if gfirst:
                                        nc.vector.tensor_copy(
                                            at[:, co:co + cols], pt[:, :cols])
                                    else:
                                        nc.vector.tensor_tensor(
                                            at[:, co:co + cols],
                                            at[:, co:co + cols],
                                            pt[:, :cols], op=OP.add)
                                op_i += 1
                        if h0end:
                            epilogue(0, SPLITC // 512)
                    epilogue(SPLITC // 512, (S_sub + 511) // 512)
                    mu = bn_head(0, l, stats, n_sub)
                    bn_apply(mu, acc512, hTs, hown_sub, S_sub, S_sub // 128)

                for l in range(L):
                    gin_sub(l)

            # ================= phase boundary =================
            with (
                tc.tile_pool(name="gres", bufs=1) as gres,
                tc.tile_pool(name="seg", bufs=2) as segp,
                tc.tile_pool(name="idx", bufs=1) as idxp,
                tc.tile_pool(name="gxs", bufs=3) as gxsp,
            ):
                hTg = gres.tile([128, S_glob], BF16, tag="hTg")
                gg_sb = gres.tile([128, TOTCHG * 8], I16, tag="ggix")
                nc.sync.dma_start(gg_sb[:], t_gg[:])

                r0_sb = idxp.tile([128, S_glob // 16], I16, tag="r0")
                r4_sb = idxp.tile([128, S_glob // 16], I16, tag="r4")
                nc.sync.dma_start(r0_sb[:], t_r0[:])
                nc.sync.dma_start(r4_sb[:], t_r4[:])
                r0b = segp.tile([128, S_glob // 128, H], BF16, tag="seg")
                r4b = segp.tile([128, S_glob // 128, H], BF16, tag="seg")
                nc.gpsimd.dma_gather(r0b[:], hown_sub[:], r0_sb[:], S_glob, S_glob, H,
                                     single_packet=False, queue_num=nextq())
                nc.gpsimd.dma_gather(r4b[:], hown_sub[:], r4_sb[:], S_glob, S_glob, H,
                                     single_packet=False, queue_num=nextq())
                inv_temp = 1.0 / float(cfg["TEMP"])
                for t in range(S_glob // 128):
                    lpt = smp.tile([128, 2], F32, tag="lpt")
                    nc.sync.dma_start(lpt[:], t_lp[t * 128:(t + 1) * 128, :])
                    d = smp.tile([128, 2], F32, tag="d")
                    nc.vector.tensor_tensor(d[:, 0:1], lpt[:, 0:1], lpt[:, 1:2], op=OP.subtract)
                    nc.scalar.activation(d[:, 0:1], d[:, 0:1], AF.Sigmoid, bias=0.0,
                                         scale=inv_temp)
                    nc.vector.tensor_scalar(d[:, 1:2], d[:, 0:1], -1.0, 1.0,
                                            op0=OP.mult, op1=OP.add)
                    hb = segp.tile([128, H], BF16, tag="hb")
                    nc.vector.tensor_scalar(hb[:], r0b[:, t, :], d[:, 0:1], None, op0=OP.mult)
                    nc.vector.scalar_tensor_tensor(hb[:], r4b[:, t, :], d[:, 1:2], hb[:],
                                                   op0=OP.mult, op1=OP.add)
                    nc.sync.dma_start(hown_glob[t * 128:(t + 1) * 128, :], hb[:])
                    pt = psT.tile([128, 128], BF16, tag="tr")
                    nc.tensor.transpose(pt[:], hb[:], idn[:])
                    nc.vector.tensor_copy(hTg[:, t * 128:(t + 1) * 128], pt[:])

                # ================= glob phase =================
                def gin_glob(l):
                    Sp, n_real, rep, hown = S_glob, n_glob, rep_glob, hown_glob
                    n_tiles = Sp // 128
                    nc.gpsimd.collective_compute(
                        "AllGather", OP.bypass, RG, [hown.ap().opt()], [rep.ap().opt()])
                    z2 = gres.tile([128, S_glob], BF16, tag="z2g")
                    stats = smp.tile([128, 64 * 6], F32, tag="stats")
                    chunks = []
                    for t in range(n_tiles):
                        for j in range(int(CtG[t])):
                            chunks.append((t, j == 0, j == int(CtG[t]) - 1))
                    psum_of = {}
                    groups = [chunks[i:i + GRP] for i in range(0, len(chunks), GRP)]
                    ch_base = 0
                    zgrp_tiles = {}

                    def run_mlp(g):
                        zg = zgrp_tiles.pop(g)
                        cols = min(512, Sp - g * 512)
                        pm = psM.tile([128, 512], F32, tag="m1")
                        nc.tensor.matmul(pm[:, :cols], W1g[:, l * H:(l + 1) * H], zg[:, :cols],
                                         start=True, stop=True)
                        z1 = zzp.tile([128, 512], BF16, tag="z1")
                        nc.scalar.activation(z1[:, :cols], pm[:, :cols], AF.Relu,
                                             bias=vcol(1, l, 0), scale=1.0)
                        pm2 = psM.tile([128, 512], F32, tag="m2")
                        nc.tensor.matmul(pm2[:, :cols], W2g[:, l * H:(l + 1) * H], z1[:, :cols],
                                         start=True, stop=True)
                        nc.vector.tensor_scalar(z2[:, g * 512:g * 512 + cols], pm2[:, :cols],
                                                vcol(1, l, 1), None, op0=OP.add)
                        realc = min(512, max(0, n_real - g * 512))
                        if realc > 0:
                            nc.vector.bn_stats(stats[:, g * 6:(g + 1) * 6],
                                               z2[:, g * 512:g * 512 + realc])

                    for gci, grp in enumerate(groups):
                        nch = len(grp)
                        xt = gxsp.tile([128, GRP, H], BF16, tag="gx")
                        n = nch * 128
                        nc.gpsimd.dma_gather(
                            xt[:, :nch, :], rep[:, :],
                            gg_sb[:, ch_base * 8:(ch_base + nch) * 8],
                            n, n, H, single_packet=False, queue_num=nextq())
                        st = gxsp.tile([128, GRP * 128], BF16, tag="gs")
                        nc.sync.dma_start(st[:, :n], t_Sglob[:, ch_base * 128:ch_base * 128 + n])
                        for j, (t, first, lastc) in enumerate(grp):
                            if first:
                                psum_of[t] = psA.tile([128, 512], F32, tag="agg",
                                                      name=f"aggg{t % 8}")
                            nc.tensor.matmul(psum_of[t][:, :128], xt[:, j, :],
                                             st[:, j * 128:(j + 1) * 128],
                                             start=first, stop=lastc)
                            if lastc:
                                g = (t * 128) // 512
                                if g not in zgrp_tiles:
                                    zgrp_tiles[g] = zzp.tile([128, 512], BF16, tag="zg",
                                                             name=f"zg{g % 4}")
                                cc = t * 128 - g * 512
                                pt = psum_of.pop(t)
                                nc.vector.scalar_tensor_tensor(
                                    zgrp_tiles[g][:, cc:cc + 128],
                                    hTg[:, t * 128:(t + 1) * 128],
                                    vcol(1, l, 4), pt[:, :128],
                                    op0=OP.mult, op1=OP.add)
                                if (t * 128 + 128) % 512 == 0 or t == n_tiles - 1:
                                    run_mlp(g)
                        ch_base += nch
                    mu = bn_head(1, l, stats, n_glob)

                    def zread(g):
                        return z2, g * 512
                    bn_apply(mu, zread, hTg, hown_glob, S_glob, n_tiles)

                for l in range(L):
                    gin_glob(l)

                # ---- readout ----
                prd = psM.tile([128, G], F32, tag="m1")
                for t in range(S_glob // 128):
                    hrow = segp.tile([128, H], BF16, tag="hrow")
                    nc.sync.dma_start(hrow[:], hown_glob[t * 128:(t + 1) * 128, :])
                    srdt = segp.tile([128, G], BF16, tag="srdt")
                    nc.sync.dma_start(srdt[:], t_Srd[:, t * G:(t + 1) * G])
                    nc.tensor.matmul(prd[:], hrow[:], srdt[:],
                                     start=(t == 0), stop=(t == S_glob // 128 - 1))
                rd_sb = gres.tile([128, G], F32, tag="rdsb")
                nc.vector.tensor_copy(rd_sb[:], prd[:])
                nc.sync.dma_start(rd_in[:], rd_sb[:])
                nc.gpsimd.collective_compute(
                    "AllReduce", OP.add, RG, [rd_in.ap().opt()], [rd_out.ap().opt()])
                rd2 = gres.tile([128, G], F32, tag="rd2")
                nc.sync.dma_start(rd2[:], rd_out[:])
                idf = gres.tile([128, 128], F32, tag="idf")
                nc.sync.dma_start(idf[:], t_idn_f[:])
                for g in range((G + 127) // 128):
                    cols = min(128, G - g * 128)
                    pt = psA.tile([128, 512], F32, tag="agg")
                    nc.tensor.transpose(pt[:cols, :128], rd2[:, g * 128:g * 128 + cols], idf[:])
                    ot = gres.tile([128, 128], F32, tag="ot")
                    nc.vector.tensor_copy(ot[:cols, :], pt[:cols, :128])
                    nc.sync.dma_start(t_out[g * 128:g * 128 + cols, :], ot[:cols, :])

    nc.compile()
    return nc


def build_inmaps(plan, weights):
    cfg = plan["cfg"]
    W, H, L = cfg["W"], cfg["H"], cfg["L"]
    n_glob, S_glob = cfg["n_glob"], cfg["S_glob"]
    lp = np.asarray(weights["log_probs"], np.float32)
    maps = []
    vecs = np.zeros((128, 10 * L), np.float32)
    for ph, pre in ((0, "sub"), (1, "glob")):
        for l in range(L):
            base = ph * 5 * L + l * 5
            vecs[:, base + 0] = np.asarray(weights[f"{pre}_b1"][l], np.float32)
            vecs[:, base + 1] = np.asarray(weights[f"{pre}_b2"][l], np.float32)
            vecs[:, base + 2] = np.asarray(weights[f"{pre}_gamma"][l], np.float32)
            vecs[:, base + 3] = np.asarray(weights[f"{pre}_beta"][l], np.float32)
            vecs[:, base + 4] = 1.0 + np.float32(weights[f"{pre}_eps"][l])
    idn = np.eye(128)
    iota = np.tile(np.arange(cfg["GRPW"], dtype=np.float32), (128, 1)).astype(F16)
    iotan = np.tile(-np.arange(cfg["GRPW"], dtype=np.float32), (128, 1)).astype(BF)
    common = {
        "W1s": np.asarray(weights["sub_W1"], np.float32).astype(BF),
        "W2s": np.asarray(weights["sub_W2"], np.float32).astype(BF),
        "W1g": np.asarray(weights["glob_W1"], np.float32).astype(BF),
        "W2g": np.asarray(weights["glob_W2"], np.float32).astype(BF),
        "vecs": vecs,
        "idnbf": idn.astype(BF),
        "idnf": idn.astype(np.float32),
        "iota16": iota,
        "iotan": iotan,
    }
    atom_bf = np.asarray(weights["atom_table"], np.float32).astype(BF)
    n_sub, S_sub = cfg["n_sub"], cfg["S_sub"]
    h0_all = atom_bf[plan["aid_ids"]]
    rep0 = np.zeros((W * S_sub, H), BF)
    for b in range(W):
        rep0[b * S_sub:b * S_sub + n_sub] = h0_all[b * n_sub:(b + 1) * n_sub]
    common["rep0"] = rep0
    for c in range(W):
        h0T = np.zeros((128, S_sub), BF)
        h0T[:, :n_sub] = h0_all[c * n_sub:(c + 1) * n_sub].T
        lpc = np.zeros((S_glob, 2), np.float32)
        lpc[:n_glob] = lp[plan["lp_sel"][c]]
        m = dict(common)
        m.update({
            "qidx": plan["sub_qidx"][c],
            "drel": plan["sub_drel"][c],
            "ggidx": plan["glob_g2_idx"][c],
            "h0T": h0T,
            "r0idx": plan["r0"][c],
            "r4idx": plan["r4"][c],
            "Sglob": plan["glob_S"][c],
            "Srd": plan["Sg"][c],
            "lp": lpc,
        })
        maps.append(m)
    return maps


def kernel(**inputs):
    import numpy as np
    cfg = dict(DEF_CFG)
    inp = {k: np.asarray(v) for k, v in inputs.items()}
    plan = build_plan(cfg, inp["x"], inp["edge_index"], inp["sub_node_map"],
                      inp["sub_edge_index"], inp["root_idx"], inp["target_batch"],
                      inp["batch"])
    nc = build_graph(plan)
    maps = build_inmaps(plan, inp)
    from concourse import bass_utils
    res = bass_utils.run_bass_kernel_spmd(nc, maps, core_ids=list(range(cfg["W"])),
                                          trace=False)
    return np.asarray(res.results[0]["out"], np.float32)


# revision 7
# speedup vs baseline: 1.0042x; 1.0038x over previous
"""Trainium2 Bass kernel: distributed GIN graph encoder on 8 NeuronCores.

v5: layer-0 replica precomputed on host (no initial AllGather); transposed
h kept resident in SBUF for both phases (no per-layer transposed DMA
reloads in epilogue/BN); S one-hot generation split between Scalar
(relu(1-(iota-d)^2)) and Vector (is_equal) engines to balance load.
Sub-phase aggregation keeps the (bucket x 256-dst-group) single-stage
gather with SBUF bf16 accumulator.
"""

import numpy as np
import ml_dtypes

BF = ml_dtypes.bfloat16
F16 = np.float16

DEF_CFG = dict(
    W=8, H=128, L=4,
    n_sub=30000, S_sub=30080,     # per-core real/padded sub rows
    n_glob=3750, S_glob=3840,
    VA=128, G=300, TEMP=0.5, BN_EPS=1e-5,
    GRPW=256,                     # dst psum group width (sub)
    CALLCH=28,                    # chunks per gather call (sub)
    GRP=16,                       # glob: chunks per gather call
    QACT=3, QCYC=4,               # S-gen quads: QACT of QCYC on Scalar, rest DVE
)


def _pack16(arr):
    """idx array (n,) int -> [128, n/16] int16 tile content (pos i -> [i%16, i//16])."""
    a = np.asarray(arr, np.int16)
    assert len(a) % 16 == 0
    t = a.reshape(-1, 16).T
    return np.tile(t, (8, 1))


def _pad128(n):
    return (n + 127) // 128 * 128


def build_plan(cfg, x, edge_index, sub_node_map, sub_edge_index, root_idx,
               target_batch, batch):
    W, H = cfg["W"], cfg["H"]
    n_sub, S_sub = cfg["n_sub"], cfg["S_sub"]
    n_glob, S_glob = cfg["n_glob"], cfg["S_glob"]
    GRPW = cfg["GRPW"]
    plan = {"cfg": cfg}
    N = n_glob * W

    # ---------- sub phase edge plan (single-stage, shared budgets) ----------
    src, dst = np.asarray(sub_edge_index[0]), np.asarray(sub_edge_index[1])
    owner = dst // n_sub
    dst_local = dst % n_sub
    src_row = (src // n_sub) * S_sub + (src % n_sub)   # row in padded replica
    ngrp = (S_sub + GRPW - 1) // GRPW

    # group edges per (core, bucket, dst-group); shared budget = max over cores
    per_core = []
    cnt = np.zeros((W, W, ngrp), np.int64)   # [core, bucket, group]
    for c in range(W):
        m = owner == c
        sc, dl = src_row[m], dst_local[m]
        b = sc // S_sub
        q = sc % S_sub
        g = dl // GRPW
        order = np.lexsort((q, g, b))
        b, q, dl, g = b[order], q[order], dl[order], g[order]
        per_core.append((b, q, dl, g))
        np.add.at(cnt[c], (b, g), 1)
    budget = _pad128(cnt.max(axis=0))        # [bucket, group] slots (multiple of 128)
    budget = np.maximum(budget, 128)
    GH = (ngrp + 1) // 2
    if (GH * GRPW) % 512:
        GH += 1          # keep the acc0/acc1 split 512-aligned for MLP groups
    nslots = int(budget.sum())
    nchunks = int((budget // 128).sum())
    # shared chunk list + calls, ordered (half, bucket, group)
    cell_off2 = np.zeros((W, ngrp), np.int64)
    off = 0
    chunks = []   # per chunk: [(g, first, stop, gfirst)]
    calls = []    # (bucket, chunk_start, nchunks, end_of_half0)
    for hh in range(2):
        glo, ghi = hh * GH, min((hh + 1) * GH, ngrp)
        for bb in range(W):
            c0 = len(chunks)
            for g in range(glo, ghi):
                cell_off2[bb, g] = off
                off += int(budget[bb, g])
                nch = int(budget[bb, g]) // 128
                for j in range(nch):
                    chunks.append([(g, j == 0, j == nch - 1, bb == 0)])
            done = 0
            nb = len(chunks) - c0
            while done < nb:
                k = min(cfg["CALLCH"], nb - done)
                done += k
                calls.append((bb, c0 + done - k, k,
                              hh == 0 and bb == W - 1 and done >= nb))
    assert off == nslots
    # per-core slot data; drel stores NEGATED dst_rel (ACT Square bias), pad -> +1
    sub_qidx, sub_drel = [], []
    for c in range(W):
        b, q, dl, g = per_core[c]
        qpad = np.zeros(nslots, np.int64)
        dpad = np.full(nslots, -1, np.int64)
        cell = b * ngrp + g
        uniq, start_idx, counts = np.unique(cell, return_index=True, return_counts=True)
        for u, s0, k in zip(uniq, start_idx, counts):
            sl = cell_off2[u // ngrp, u % ngrp] + np.arange(k)
            qpad[sl] = q[s0:s0 + k]
            dpad[sl] = dl[s0:s0 + k] - (u % ngrp) * GRPW
        sub_qidx.append(_pack16(qpad))
        sub_drel.append((-dpad.reshape(-1, 128).T).astype(np.float32))  # [128, nchunks]
    plan["sub_nslots"] = nslots
    plan["sub_nops"] = nchunks
    plan["sub_qidx"] = sub_qidx
    plan["sub_drel"] = sub_drel
    plan["sub_chunks"] = chunks
    plan["sub_calls"] = calls
    plan["sub_ngrp"] = ngrp
    plan["sub_GH"] = GH

    # ---------- glob phase edge plan ----------
    gsrc, gdst = np.asarray(edge_index[0]), np.asarray(edge_index[1])
    gowner = gdst // n_glob
    gdst_local = gdst % n_glob
    gsrc_row = (gsrc // n_glob) * S_glob + (gsrc % n_glob)
    n_tiles_glob = S_glob // 128
    per_core_g = []
    CtG = np.zeros(n_tiles_glob, np.int64)
    for c in range(W):
        m = gowner == c
        sc, dl = gsrc_row[m], gdst_local[m]
        order = np.argsort(dl, kind="stable")
        sc, dl = sc[order], dl[order]
        per_core_g.append((sc, dl))
        gcnt = np.bincount(dl // 128, minlength=n_tiles_glob)
        CtG = np.maximum(CtG, (gcnt + 127) // 128)
    CtG = np.maximum(CtG, 1)
    plan["glob_Ct"] = CtG
    TOTCHG = int(CtG.sum())
    plan["glob_TOTCH"] = TOTCHG
    tile_slot_off_g = np.zeros(n_tiles_glob + 1, np.int64)
    tile_slot_off_g[1:] = np.cumsum(CtG * 128)
    gg_idx_cores, Sg_cores = [], []
    for c in range(W):
        sc, dl = per_core_g[c]
        g2 = np.zeros(TOTCHG * 128, np.int64)
        S = np.zeros((TOTCHG * 128, 128), BF)
        tl = dl // 128
        for t in range(n_tiles_glob):
            mt = tl == t
            k = int(mt.sum())
            slots = tile_slot_off_g[t] + np.arange(k)
            g2[slots] = sc[mt]
            S[slots, dl[mt] % 128] = BF(1.0)
        gg_idx_cores.append(_pack16(g2))
        Sg_cores.append(S)
    plan["glob_g2_idx"] = gg_idx_cores
    plan["glob_S"] = [S.reshape(TOTCHG, 128, 128).transpose(1, 0, 2).reshape(128, TOTCHG * 128)
                      for S in Sg_cores]

    # ---------- atom encode (host) ----------
    plan["aid_ids"] = np.asarray(x)[np.asarray(sub_node_map)]

    # ---------- phase boundary (roots) ----------
    tb_arr = np.asarray(target_batch)
    ri = np.asarray(root_idx)
    order = np.argsort(tb_arr, kind="stable")
    assert (np.bincount(tb_arr, minlength=N) == 2).all(), "need exactly 2 roots/node"
    r_sorted = ri[order].reshape(N, 2)
    lp_order = order.reshape(N, 2)
    plan["r0"], plan["r4"], plan["lp_sel"] = [], [], []
    for c in range(W):
        r = r_sorted[c * n_glob:(c + 1) * n_glob]
        lo = c * n_sub
        assert ((r >= lo) & (r < lo + n_sub)).all(), "roots must be core-local"
        r0 = np.zeros(S_glob, np.int64)
        r4 = np.zeros(S_glob, np.int64)
        r0[:n_glob] = r[:, 0] - lo
        r4[:n_glob] = r[:, 1] - lo
        plan["r0"].append(_pack16(r0))
        plan["r4"].append(_pack16(r4))
        plan["lp_sel"].append(lp_order[c * n_glob:(c + 1) * n_glob])

    # ---------- readout ----------
    b_arr = np.asarray(batch)
    plan["Sg"] = []
    for c in range(W):
        Srd = np.zeros((S_glob, cfg["G"]), BF)
        ids = b_arr[c * n_glob:(c + 1) * n_glob]
        Srd[np.arange(n_glob), ids] = BF(1.0)
        nt = S_glob // 128
        plan["Sg"].append(Srd.reshape(nt, 128, cfg["G"]).transpose(1, 0, 2).reshape(128, nt * cfg["G"]))
    return plan


def _install_queue_aware_lanes():
    """Make Tile's DMASW lane assignment queue-aware: lane = queue*2 + rr."""
    import concourse.tile_sem_assignment as tsa
    if getattr(tsa, "_qaware_installed", False):
        return
    orig = tsa.TileClockTick._assign_tick
    import concourse.mybir as mb

    def patched(self, inst):
        qn = getattr(inst, "queue_num", None)
        if (qn is not None and inst.engine == mb.EngineType.Pool
                and isinstance(inst, tsa.DMAInst)
                and self.swdge_sem_count == 8):
            rr_map = getattr(self, "_q_rr", None)
            if rr_map is None:
                rr_map = self._q_rr = {}
            sub = rr_map.get(qn, 0)
            rr_map[qn] = (sub + 1) % 2
            lane = qn * 2 + sub
            save = self.next_sw_dma_idx
            self.next_sw_dma_idx = lane
            try:
                return orig(self, inst)
            finally:
                self.next_sw_dma_idx = save
        return orig(self, inst)

    tsa.TileClockTick._assign_tick = patched
    tsa._qaware_installed = True


def build_graph(plan):
    from concourse import bass, mybir, bacc
    import concourse.tile as tile

    cfg = plan["cfg"]
    W, H, L = cfg["W"], cfg["H"], cfg["L"]
    n_sub, S_sub = cfg["n_sub"], cfg["S_sub"]
    n_glob, S_glob = cfg["n_glob"], cfg["S_glob"]
    G = cfg["G"]
    GRPW = cfg["GRPW"]
    BF16 = mybir.dt.bfloat16
    FP16 = mybir.dt.float16
    F32 = mybir.dt.float32
    I16 = mybir.dt.int16
    AF = mybir.ActivationFunctionType
    OP = mybir.AluOpType
    GRP = cfg["GRP"]
    ngrp = plan["sub_ngrp"]
    QACT, QCYC = cfg["QACT"], cfg["QCYC"]

    _install_queue_aware_lanes()
    nc = bacc.Bacc("TRN2", target_bir_lowering=False, debug=False, num_devices=W,
                   num_swdge_queues=4)

    def inp(name, shape, dt):
        return nc.dram_tensor(name, shape, dt, kind="ExternalInput")

    TOTCHG = plan["glob_TOTCH"]
    nslots, nops = plan["sub_nslots"], plan["sub_nops"]
    t_qidx = inp("qidx", [128, nslots // 16], I16)
    t_drel = inp("drel", [128, nops], F32)
    t_iota = inp("iota16", [128, GRPW], FP16)
    t_iotan = inp("iotan", [128, GRPW], BF16)
    t_gg = inp("ggidx", [128, TOTCHG * 8], I16)
    t_rep0 = inp("rep0", [W * S_sub, H], BF16)
    t_h0T = inp("h0T", [128, S_sub], BF16)
    t_r0 = inp("r0idx", [128, S_glob // 16], I16)
    t_r4 = inp("r4idx", [128, S_glob // 16], I16)
    t_Sglob = inp("Sglob", [128, TOTCHG * 128], BF16)
    t_Srd = inp("Srd", [128, (S_glob // 128) * G], BF16)
    t_idn_bf = inp("idnbf", [128, 128], BF16)
    t_idn_f = inp("idnf", [128, 128], F32)
    t_W1s = inp("W1s", [L, H, H], BF16)
    t_W2s = inp("W2s", [L, H, H], BF16)
    t_W1g = inp("W1g", [L, H, H], BF16)
    t_W2g = inp("W2g", [L, H, H], BF16)
    t_vecs = inp("vecs", [128, 10 * L], F32)
    t_lp = inp("lp", [S_glob, 2], F32)
    t_out = nc.dram_tensor("out", [G, H], F32, kind="ExternalOutput")

    rep_sub = nc.dram_tensor("rep_sub", [W * S_sub, H], BF16, addr_space="Shared")
    rep_glob = nc.dram_tensor("rep_glob", [W * S_glob, H], BF16, addr_space="Shared")
    hown_sub = nc.dram_tensor("hown_sub", [S_sub, H], BF16)
    hown_glob = nc.dram_tensor("hown_glob", [S_glob, H], BF16)
    ar_in = nc.dram_tensor("ar_in", [128, 2], F32)
    ar_out = nc.dram_tensor("ar_out", [128, 2], F32, addr_space="Shared")
    rd_in = nc.dram_tensor("rd_in", [128, G], F32)
    rd_out = nc.dram_tensor("rd_out", [128, G], F32, addr_space="Shared")

    RG = [list(range(W))]
    _qrr = [0]

    def nextq():
        q = (_qrr[0] % 8) // 2
        _qrr[0] += 1
        return q

    CtG = plan["glob_Ct"]
    chunks_meta = plan["sub_chunks"]
    calls_meta = plan["sub_calls"]
    GH = plan["sub_GH"]
    SPLITC = GH * GRPW

    with tile.TileContext(nc) as tc:
        with (
            tc.tile_pool(name="const", bufs=1) as constp,
            tc.tile_pool(name="xs", bufs=3) as xsp,
            tc.tile_pool(name="sgen", bufs=2) as sgp,
            tc.tile_pool(name="zz", bufs=3) as zzp,
            tc.tile_pool(name="small", bufs=2) as smp,
            tc.tile_pool(name="stg", bufs=2) as stgp,
            tc.tile_pool(name="psA", bufs=2, space="PSUM") as psA,
            tc.tile_pool(name="psM", bufs=1, space="PSUM") as psM,
            tc.tile_pool(name="psT", bufs=2, space="PSUM") as psT,
        ):
            # ---- constants resident ----
            vecs = constp.tile([128, 10 * L], F32)
            nc.sync.dma_start(vecs[:], t_vecs[:])
            W1s = constp.tile([128, L * H], BF16)
            W2s = constp.tile([128, L * H], BF16)
            W1g = constp.tile([128, L * H], BF16)
            W2g = constp.tile([128, L * H], BF16)
            for l in range(L):
                nc.sync.dma_start(W1s[:, l * H:(l + 1) * H], t_W1s[l])
                nc.sync.dma_start(W2s[:, l * H:(l + 1) * H], t_W2s[l])
                nc.sync.dma_start(W1g[:, l * H:(l + 1) * H], t_W1g[l])
                nc.sync.dma_start(W2g[:, l * H:(l + 1) * H], t_W2g[l])
            idn = constp.tile([128, 128], BF16, tag="idn")
            nc.sync.dma_start(idn[:], t_idn_bf[:])
            qidx_sb = constp.tile([128, nslots // 16], I16, tag="qidx")
            nc.sync.dma_start(qidx_sb[:], t_qidx[:])
            drel_sb = constp.tile([128, nops], F32, tag="drel")
            nc.sync.dma_start(drel_sb[:], t_drel[:])
            iota_sb = constp.tile([128, GRPW], FP16, tag="iota")
            nc.sync.dma_start(iota_sb[:], t_iota[:])
            iotan_sb = constp.tile([128, GRPW], BF16, tag="iotan")
            nc.sync.dma_start(iotan_sb[:], t_iotan[:])
            # zero tiles: let DVE ops that overlap gather descgen use 2-input
            # op classes (tensor_tensor / scalar_tensor_tensor), which never
            # grab the shared SBUF port pair that SWDGE descriptor generation
            # needs. tensor_scalar/copy would block GpSimd for their duration.
            zero_b = constp.tile([128, GRPW], BF16, tag="zerob")
            nc.vector.memset(zero_b[:], 0.0)
            zero_f = constp.tile([128, 512], F32, tag="zerof")
            nc.vector.memset(zero_f[:], 0.0)

            def vcol(phase, l, j):
                return vecs[:, (phase * 5 * L + l * 5 + j):(phase * 5 * L + l * 5 + j) + 1]

            # shared BN scalar computation (stats -> mu tile) ---------------
            def bn_head(phase, l, stats, n_real):
                ngr = (n_real + 511) // 512
                mv = smp.tile([128, 2], F32, tag="mv")
                nc.vector.bn_aggr(mv[:], stats[:, :ngr * 6])
                sin = smp.tile([128, 2], F32, tag="sin")
                nc.vector.tensor_tensor(sin[:, 1:2], mv[:, 0:1], mv[:, 0:1], op=OP.mult)
                nc.vector.tensor_tensor(sin[:, 1:2], sin[:, 1:2], mv[:, 1:2], op=OP.add)
                nc.vector.tensor_copy(sin[:, 0:1], mv[:, 0:1])
                nc.sync.dma_start(ar_in[:], sin[:])
                nc.gpsimd.collective_compute(
                    "AllReduce", OP.add, RG, [ar_in.ap().opt()], [ar_out.ap().opt()])
                sg = smp.tile([128, 2], F32, tag="sg")
                nc.sync.dma_start(sg[:], ar_out[:])
                mu = smp.tile([128, 4], F32, tag="mu")
                nc.scalar.mul(mu[:, 0:1], sg[:, 0:1], 1.0 / W)
                nc.scalar.mul(mu[:, 1:2], sg[:, 1:2], 1.0 / W)
                nc.vector.tensor_tensor(mu[:, 2:3], mu[:, 0:1], mu[:, 0:1], op=OP.mult)
                nc.vector.tensor_tensor(mu[:, 1:2], mu[:, 1:2], mu[:, 2:3], op=OP.subtract)
                nc.vector.tensor_scalar(mu[:, 1:2], mu[:, 1:2], float(cfg["BN_EPS"]), None,
                                        op0=OP.add)
                nc.scalar.activation(mu[:, 1:2], mu[:, 1:2], AF.Sqrt, bias=0.0, scale=1.0)
                nc.vector.reciprocal(mu[:, 1:2], mu[:, 1:2])
                nc.vector.tensor_tensor(mu[:, 2:3], vcol(phase, l, 2), mu[:, 1:2], op=OP.mult)
                nc.vector.tensor_tensor(mu[:, 3:4], mu[:, 0:1], mu[:, 2:3], op=OP.mult)
                nc.vector.tensor_tensor(mu[:, 3:4], vcol(phase, l, 3), mu[:, 3:4], op=OP.subtract)
                return mu

            # normalize z (SBUF, h^T layout) + residual into hT, write hown
            def bn_apply(mu, zread, hT, hown, Sp, n_tiles):
                stgt = stgp.tile([128, 16, 128], BF16, tag="stg")
                stg_fill = 0
                stg_t0 = 0
                for g in range((Sp + 511) // 512):
                    base = g * 512
                    cols = min(512, Sp - base)
                    zt_, co_ = zread(g)
                    hn = zzp.tile([128, 512], BF16, tag="hn")
                    nc.vector.tensor_scalar(hn[:, :cols], zt_[:, co_:co_ + cols],
                                            mu[:, 2:3], mu[:, 3:4],
                                            op0=OP.mult, op1=OP.add)
                    nc.vector.tensor_tensor(hT[:, base:base + cols], hn[:, :cols],
                                            hT[:, base:base + cols], op=OP.add)
                    for q in range(cols // 128):
                        t = base // 128 + q
                        pt = psT.tile([128, 128], BF16, tag="tr")
                        nc.tensor.transpose(pt[:], hT[:, t * 128:(t + 1) * 128], idn[:])
                        nc.vector.tensor_copy(stgt[:, stg_fill, :], pt[:])
                        stg_fill += 1
                        if stg_fill == 16 or t == n_tiles - 1:
                            nc.sync.dma_start(
                                hown.ap().rearrange("(c p) h -> p c h", p=128)[:, stg_t0:stg_t0 + stg_fill, :],
                                stgt[:, :stg_fill, :])
                            stg_t0 += stg_fill
                            stg_fill = 0
                            if t != n_tiles - 1:
                                stgt = stgp.tile([128, 16, 128], BF16, tag="stg")

            # ================= sub phase =================
            with tc.tile_pool(name="subres", bufs=1) as subres:
                hTs = subres.tile([128, S_sub], BF16, tag="hTs")
                nc.sync.dma_start(hTs[:], t_h0T[:])

                def gin_sub(l):
                    if l > 0:
                        nc.gpsimd.collective_compute(
                            "AllGather", OP.bypass, RG,
                            [hown_sub.ap().opt()], [rep_sub.ap().opt()])
                        rep_src = rep_sub
                    else:
                        rep_src = t_rep0
                    acc0 = subres.tile([128, SPLITC], BF16, tag="acc0")
                    acc1 = subres.tile([128, S_sub - SPLITC], BF16, tag="acc1")
                    stats = smp.tile([128, 64 * 6], F32, tag="stats")

                    def acc_at(g):
                        if g < GH:
                            return acc0, g * GRPW
                        return acc1, g * GRPW - SPLITC

                    def acc512(g):
                        co = g * 512
                        if co < SPLITC:
                            return acc0, co
                        return acc1, co - SPLITC

                    def epilogue(g0, g1):
                        for g in range(g0, g1):
                            cols = min(512, S_sub - g * 512)
                            at, co = acc512(g)
                            zg = zzp.tile([128, 512], BF16, tag="zg2")
                            nc.vector.scalar_tensor_tensor(
                                zg[:, :cols], hTs[:, g * 512:g * 512 + cols],
                                vcol(0, l, 4), at[:, co:co + cols],
                                op0=OP.mult, op1=OP.add)
                            pm = psM.tile([128, 512], F32, tag="m1")
                            nc.tensor.matmul(pm[:, :cols], W1s[:, l * H:(l + 1) * H],
                                             zg[:, :cols], start=True, stop=True)
                            z1 = zzp.tile([128, 512], BF16, tag="z1")
                            nc.scalar.activation(z1[:, :cols], pm[:, :cols], AF.Relu,
                                                 bias=vcol(0, l, 0), scale=1.0)
                            pm2 = psM.tile([128, 512], F32, tag="m2")
                            nc.tensor.matmul(pm2[:, :cols], W2s[:, l * H:(l + 1) * H],
                                             z1[:, :cols], start=True, stop=True)
                            nc.vector.scalar_tensor_tensor(
                                at[:, co:co + cols], pm2[:, :cols],
                                vcol(0, l, 1), zero_f[:, :cols],
                                op0=OP.add, op1=OP.add)
                            realc = min(512, max(0, n_sub - g * 512))
                            if realc > 0:
                                nc.vector.bn_stats(stats[:, g * 6:(g + 1) * 6],
                                                   at[:, co:co + realc])

                    op_i = 0
                    ps_of = {}
                    for (bb, c0, k, h0end) in calls_meta:
                        xt = xsp.tile([128, cfg["CALLCH"], H], BF16, tag="x")
                        n = k * 128
                        nc.gpsimd.dma_gather(
                            xt[:, :k, :], rep_src[bb * S_sub:(bb + 1) * S_sub, :],
                            qidx_sb[:, (c0 * 128) // 16:(c0 * 128 + n) // 16], n, n, H,
                            single_packet=False, queue_num=nextq())
                        # S-gen: quads on Scalar (Square+fused Relu), rest on
                        # DVE (is_equal against negated iota).
                        St4_of = {}
                        for jq in range(0, k, 4):
                            kq = min(4, k - jq)
                            if (jq // 4) % QCYC < QACT:
                                sq4 = sgp.tile([128, 4 * GRPW], BF16, tag="sgq")
                                for t in range(kq):
                                    nc.scalar.activation(
                                        sq4[:, t * GRPW:(t + 1) * GRPW], iota_sb[:],
                                        AF.Square,
                                        bias=drel_sb[:, op_i + jq + t:op_i + jq + t + 1],
                                        scale=1.0)
                                St4 = sgp.tile([128, 4 * GRPW], BF16, tag="sgen")
                                nc.scalar.activation(St4[:, :kq * GRPW], sq4[:, :kq * GRPW],
                                                     AF.Relu, bias=1.0, scale=-1.0)
                                for t in range(kq):
                                    St4_of[jq + t] = (St4, t * GRPW)
                            else:
                                for t in range(kq):
                                    sd = sgp.tile([128, GRPW], BF16, tag="sgd", bufs=6)
                                    nc.vector.scalar_tensor_tensor(
                                        sd[:], iotan_sb[:],
                                        drel_sb[:, op_i + jq + t:op_i + jq + t + 1],
                                        zero_b[:], op0=OP.is_equal, op1=OP.add)
                                    St4_of[jq + t] = (sd, 0)
                        for j in range(k):
                            ops = chunks_meta[c0 + j]
                            for (g, first, stop, gfirst) in ops:
                                St4, sco = St4_of[j]
                                if first:
                                    ps_of[g] = psA.tile([128, GRPW], F32, tag="agg2",
                                                        name=f"agg{g % 2}")
                                cols = min(GRPW, S_sub - g * GRPW)
                                nc.tensor.matmul(ps_of[g][:, :cols], xt[:, j, :],
                                                 St4[:, sco:sco + cols],
                                                 start=first, stop=stop)
                                if stop:
                                    pt = ps_of.pop(g)
                                    at, co = acc_at(g)
                                    if gfirst:
                                        nc.vector.tensor_tensor(
                                            at[:, co:co + cols], pt[:, :cols],
                                            zero_f[:, :cols], op=OP.add)
                                    else:
                                        nc.vector.tensor_tensor(
                                            at[:, co:co + cols],
                                            at[:, co:co + cols],
                                            pt[:, :cols], op=OP.add)
                                op_i += 1
                        if h0end:
                            epilogue(0, SPLITC // 512)
                    epilogue(SPLITC // 512, (S_sub + 511) // 512)
                    mu = bn_head(0, l, stats, n_sub)
                    bn_apply(mu, acc512, hTs, hown_sub, S_sub, S_sub // 128)

                for l in range(L):
                    gin_sub(l)

            # ================= phase boundary =================
            with (
                tc.tile_pool(name="gres", bufs=1) as gres,
                tc.tile_pool(name="seg", bufs=2) as segp,
                tc.tile_pool(name="idx", bufs=1) as idxp,
                tc.tile_pool(name="gxs", bufs=3) as gxsp,
            ):
                hTg = gres.tile([128, S_glob], BF16, tag="hTg")
                gg_sb = gres.tile([128, TOTCHG * 8], I16, tag="ggix")
                nc.sync.dma_start(gg_sb[:], t_gg[:])

                r0_sb = idxp.tile([128, S_glob // 16], I16, tag="r0")
                r4_sb = idxp.tile([128, S_glob // 16], I16, tag="r4")
                nc.sync.dma_start(r0_sb[:], t_r0[:])
                nc.sync.dma_start(r4_sb[:], t_r4[:])
                r0b = segp.tile([128, S_glob // 128, H], BF16, tag="seg")
                r4b = segp.tile([128, S_glob // 128, H], BF16, tag="seg")
                nc.gpsimd.dma_gather(r0b[:], hown_sub[:], r0_sb[:], S_glob, S_glob, H,
                                     single_packet=False, queue_num=nextq())
                nc.gpsimd.dma_gather(r4b[:], hown_sub[:], r4_sb[:], S_glob, S_glob, H,
                                     single_packet=False, queue_num=nextq())
                inv_temp = 1.0 / float(cfg["TEMP"])
                for t in range(S_glob // 128):
                    lpt = smp.tile([128, 2], F32, tag="lpt")
                    nc.sync.dma_start(lpt[:], t_lp[t * 128:(t + 1) * 128, :])
                    d = smp.tile([128, 2], F32, tag="d")
                    nc.vector.tensor_tensor(d[:, 0:1], lpt[:, 0:1], lpt[:, 1:2], op=OP.subtract)
                    nc.scalar.activation(d[:, 0:1], d[:, 0:1], AF.Sigmoid, bias=0.0,
                                         scale=inv_temp)
                    nc.vector.tensor_scalar(d[:, 1:2], d[:, 0:1], -1.0, 1.0,
                                            op0=OP.mult, op1=OP.add)
                    hb = segp.tile([128, H], BF16, tag="hb")
                    nc.vector.tensor_scalar(hb[:], r0b[:, t, :], d[:, 0:1], None, op0=OP.mult)
                    nc.vector.scalar_tensor_tensor(hb[:], r4b[:, t, :], d[:, 1:2], hb[:],
                                                   op0=OP.mult, op1=OP.add)
                    nc.sync.dma_start(hown_glob[t * 128:(t + 1) * 128, :], hb[:])
                    pt = psT.tile([128, 128], BF16, tag="tr")
                    nc.tensor.transpose(pt[:], hb[:], idn[:])
                    nc.vector.tensor_copy(hTg[:, t * 128:(t + 1) * 128], pt[:])

                # ================= glob phase =================
                def gin_glob(l):
                    Sp, n_real, rep, hown = S_glob, n_glob, rep_glob, hown_glob
                    n_tiles = Sp // 128
                    nc.gpsimd.collective_compute(
                        "AllGather", OP.bypass, RG, [hown.ap().opt()], [rep.ap().opt()])
                    z2 = gres.tile([128, S_glob], BF16, tag="z2g")
                    stats = smp.tile([128, 64 * 6], F32, tag="stats")
                    chunks = []
                    for t in range(n_tiles):
                        for j in range(int(CtG[t])):
                            chunks.append((t, j == 0, j == int(CtG[t]) - 1))
                    psum_of = {}
                    groups = [chunks[i:i + GRP] for i in range(0, len(chunks), GRP)]
                    ch_base = 0
                    zgrp_tiles = {}

                    def run_mlp(g):
                        zg = zgrp_tiles.pop(g)
                        cols = min(512, Sp - g * 512)
                        pm = psM.tile([128, 512], F32, tag="m1")
                        nc.tensor.matmul(pm[:, :cols], W1g[:, l * H:(l + 1) * H], zg[:, :cols],
                                         start=True, stop=True)
                        z1 = zzp.tile([128, 512], BF16, tag="z1")
                        nc.scalar.activation(z1[:, :cols], pm[:, :cols], AF.Relu,
                                             bias=vcol(1, l, 0), scale=1.0)
                        pm2 = psM.tile([128, 512], F32, tag="m2")
                        nc.tensor.matmul(pm2[:, :cols], W2g[:, l * H:(l + 1) * H], z1[:, :cols],
                                         start=True, stop=True)
                        nc.vector.scalar_tensor_tensor(
                            z2[:, g * 512:g * 512 + cols], pm2[:, :cols],
                            vcol(1, l, 1), zero_f[:, :cols],
                            op0=OP.add, op1=OP.add)
                        realc = min(512, max(0, n_real - g * 512))
                        if realc > 0:
                            nc.vector.bn_stats(stats[:, g * 6:(g + 1) * 6],
                                               z2[:, g * 512:g * 512 + realc])

                    for gci, grp in enumerate(groups):
                        nch = len(grp)
                        xt = gxsp.tile([128, GRP, H], BF16, tag="gx")
                        n = nch * 128
                        nc.gpsimd.dma_gather(
                            xt[:, :nch, :], rep[:, :],
                            gg_sb[:, ch_base * 8:(ch_base + nch) * 8],
                            n, n, H, single_packet=False, queue_num=nextq())
                        st = gxsp.tile([128, GRP * 128], BF16, tag="gs")
                        nc.sync.dma_start(st[:, :n], t_Sglob[:, ch_base * 128:ch_base * 128 + n])
                        for j, (t, first, lastc) in enumerate(grp):
                            if first:
                                psum_of[t] = psA.tile([128, 512], F32, tag="agg",
                                                      name=f"aggg{t % 8}")
                            nc.tensor.matmul(psum_of[t][:, :128], xt[:, j, :],
                                             st[:, j * 128:(j + 1) * 128],
                                             start=first, stop=lastc)
                            if lastc:
                                g = (t * 128) // 512
                                if g not in zgrp_tiles:
                                    zgrp_tiles[g] = zzp.tile([128, 512], BF16, tag="zg",
                                                             name=f"zg{g % 4}")
                                cc = t * 128 - g * 512
                                pt = psum_of.pop(t)
                                nc.vector.scalar_tensor_tensor(
                                    zgrp_tiles[g][:, cc:cc + 128],
                                    hTg[:, t * 128:(t + 1) * 128],
                                    vcol(1, l, 4), pt[:, :128],
                                    op0=OP.mult, op1=OP.add)
                                if (t * 128 + 128) % 512 == 0 or t == n_tiles - 1:
                                    run_mlp(g)
                        ch_base += nch
                    mu = bn_head(1, l, stats, n_glob)

                    def zread(g):
                        return z2, g * 512
                    bn_apply(mu, zread, hTg, hown_glob, S_glob, n_tiles)

                for l in range(L):
                    gin_glob(l)

                # ---- readout ----
                prd = psM.tile([128, G], F32, tag="m1")
                for t in range(S_glob // 128):
                    hrow = segp.tile([128, H], BF16, tag="hrow")
                    nc.sync.dma_start(hrow[:], hown_glob[t * 128:(t + 1) * 128, :])
                    srdt = segp.tile([128, G], BF16, tag="srdt")
                    nc.sync.dma_start(srdt[:], t_Srd[:, t * G:(t + 1) * G])
                    nc.tensor.matmul(prd[:], hrow[:], srdt[:],
                                     start=(t == 0), stop=(t == S_glob // 128 - 1))
                rd_sb = gres.tile([128, G], F32, tag="rdsb")
                nc.vector.tensor_copy(rd_sb[:], prd[:])
                nc.sync.dma_start(rd_in[:], rd_sb[:])
                nc.gpsimd.collective_compute(
                    "AllReduce", OP.add, RG, [rd_in.ap().opt()], [rd_out.ap().opt()])
                rd2 = gres.tile([128, G], F32, tag="rd2")
                nc.sync.dma_start(rd2[:], rd_out[:])
                idf = gres.tile([128, 128], F32, tag="idf")
                nc.sync.dma_start(idf[:], t_idn_f[:])
                for g in range((G + 127) // 128):
                    cols = min(128, G - g * 128)
                    pt = psA.tile([128, 512], F32, tag="agg")
                    nc.tensor.transpose(pt[:cols, :128], rd2[:, g * 128:g * 128 + cols], idf[:])
                    ot = gres.tile([128, 128], F32, tag="ot")
                    nc.vector.tensor_copy(ot[:cols, :], pt[:cols, :128])
                    nc.sync.dma_start(t_out[g * 128:g * 128 + cols, :], ot[:cols, :])

    nc.compile()
    return nc


def build_inmaps(plan, weights):
    cfg = plan["cfg"]
    W, H, L = cfg["W"], cfg["H"], cfg["L"]
    n_glob, S_glob = cfg["n_glob"], cfg["S_glob"]
    lp = np.asarray(weights["log_probs"], np.float32)
    maps = []
    vecs = np.zeros((128, 10 * L), np.float32)
    for ph, pre in ((0, "sub"), (1, "glob")):
        for l in range(L):
            base = ph * 5 * L + l * 5
            vecs[:, base + 0] = np.asarray(weights[f"{pre}_b1"][l], np.float32)
            vecs[:, base + 1] = np.asarray(weights[f"{pre}_b2"][l], np.float32)
            vecs[:, base + 2] = np.asarray(weights[f"{pre}_gamma"][l], np.float32)
            vecs[:, base + 3] = np.asarray(weights[f"{pre}_beta"][l], np.float32)
            vecs[:, base + 4] = 1.0 + np.float32(weights[f"{pre}_eps"][l])
    idn = np.eye(128)
    iota = np.tile(np.arange(cfg["GRPW"], dtype=np.float32), (128, 1)).astype(F16)
    iotan = np.tile(-np.arange(cfg["GRPW"], dtype=np.float32), (128, 1)).astype(BF)
    common = {
        "W1s": np.asarray(weights["sub_W1"], np.float32).astype(BF),
        "W2s": np.asarray(weights["sub_W2"], np.float32).astype(BF),
        "W1g": np.asarray(weights["glob_W1"], np.float32).astype(BF),
        "W2g": np.asarray(weights["glob_W2"], np.float32).astype(BF),
        "vecs": vecs,
        "idnbf": idn.astype(BF),
        "idnf": idn.astype(np.float32),
        "iota16": iota,
        "iotan": iotan,
    }
    atom_bf = np.asarray(weights["atom_table"], np.float32).astype(BF)
    n_sub, S_sub = cfg["n_sub"], cfg["S_sub"]
    h0_all = atom_bf[plan["aid_ids"]]
    rep0 = np.zeros((W * S_sub, H), BF)
    for b in range(W):
        rep0[b * S_sub:b * S_sub + n_sub] = h0_all[b * n_sub:(b + 1) * n_sub]
    common["rep0"] = rep0
    for c in range(W):
        h0T = np.zeros((128, S_sub), BF)
        h0T[:, :n_sub] = h0_all[c * n_sub:(c + 1) * n_sub].T
        lpc = np.zeros((S_glob, 2), np.float32)
        lpc[:n_glob] = lp[plan["lp_sel"][c]]
        m = dict(common)
        m.update({
            "qidx": plan["sub_qidx"][c],
            "drel": plan["sub_drel"][c],
            "ggidx": plan["glob_g2_idx"][c],
            "h0T": h0T,
            "r0idx": plan["r0"][c],
            "r4idx": plan["r4"][c],
            "Sglob": plan["glob_S"][c],
            "Srd": plan["Sg"][c],
            "lp": lpc,
        })
        maps.append(m)
    return maps


def kernel(**inputs):
    import numpy as np
    cfg = dict(DEF_CFG)
    inp = {k: np.asarray(v) for k, v in inputs.items()}
    plan = build_plan(cfg, inp["x"], inp["edge_index"], inp["sub_node_map"],
                      inp["sub_edge_index"], inp["root_idx"], inp["target_batch"],
                      inp["batch"])
    nc = build_graph(plan)
    maps = build_inmaps(plan, inp)
    from concourse import bass_utils
    res = bass_utils.run_bass_kernel_spmd(nc, maps, core_ids=list(range(cfg["W"])),
                                          trace=False)
    return np.asarray(res.results[0]["out"], np.float32)


# revision 14
# speedup vs baseline: 1.1113x; 1.1067x over previous
"""Trainium2 Bass kernel: distributed GIN graph encoder on 8 NeuronCores.

v5: layer-0 replica precomputed on host (no initial AllGather); transposed
h kept resident in SBUF for both phases (no per-layer transposed DMA
reloads in epilogue/BN); S one-hot generation split between Scalar
(relu(1-(iota-d)^2)) and Vector (is_equal) engines to balance load.
Sub-phase aggregation keeps the (bucket x 256-dst-group) single-stage
gather with SBUF bf16 accumulator.
"""

import numpy as np
import ml_dtypes

BF = ml_dtypes.bfloat16
F16 = np.float16

DEF_CFG = dict(
    W=8, H=128, L=4,
    n_sub=30000, S_sub=30080,     # per-core real/padded sub rows
    n_glob=3750, S_glob=3840,
    VA=128, G=300, TEMP=0.5, BN_EPS=1e-5,
    GRPW=256,                     # dst psum group width (sub)
    CALLCH=32,                    # chunks per gather call (sub)
    GRP=32,                       # glob: chunks per gather call
    QACT=4, QCYC=4,               # S-gen quads: QACT of QCYC on Scalar, rest DVE
)


def _pack16(arr):
    """idx array (n,) int -> [128, n/16] int16 tile content (pos i -> [i%16, i//16])."""
    a = np.asarray(arr, np.int16)
    assert len(a) % 16 == 0
    t = a.reshape(-1, 16).T
    return np.tile(t, (8, 1))


def _pad128(n):
    return (n + 127) // 128 * 128


def build_plan(cfg, x, edge_index, sub_node_map, sub_edge_index, root_idx,
               target_batch, batch):
    W, H = cfg["W"], cfg["H"]
    n_sub, S_sub = cfg["n_sub"], cfg["S_sub"]
    n_glob, S_glob = cfg["n_glob"], cfg["S_glob"]
    GRPW = cfg["GRPW"]
    plan = {"cfg": cfg}
    N = n_glob * W

    # ---------- sub phase edge plan (single-stage, shared budgets) ----------
    src, dst = np.asarray(sub_edge_index[0]), np.asarray(sub_edge_index[1])
    owner = dst // n_sub
    dst_local = dst % n_sub
    src_row = (src // n_sub) * S_sub + (src % n_sub)   # row in padded replica
    ngrp = (S_sub + GRPW - 1) // GRPW

    # group edges per (core, bucket, dst-group); shared budget = max over cores
    per_core = []
    cnt = np.zeros((W, W, ngrp), np.int64)   # [core, bucket, group]
    for c in range(W):
        m = owner == c
        sc, dl = src_row[m], dst_local[m]
        b = sc // S_sub
        q = sc % S_sub
        g = dl // GRPW
        order = np.lexsort((q, g, b))
        b, q, dl, g = b[order], q[order], dl[order], g[order]
        per_core.append((b, q, dl, g))
        np.add.at(cnt[c], (b, g), 1)
    budget = _pad128(cnt.max(axis=0))        # [bucket, group] slots (multiple of 128)
    budget = np.maximum(budget, 128)
    GH = (ngrp + 1) // 2
    if (GH * GRPW) % 512:
        GH += 1          # keep the acc0/acc1 split 512-aligned for MLP groups
    nslots = int(budget.sum())
    nchunks = int((budget // 128).sum())
    # shared chunk list + calls, ordered (half, bucket, group)
    cell_off2 = np.zeros((W, ngrp), np.int64)
    off = 0
    chunks = []   # per chunk: [(g, first, stop, gfirst)]
    calls = []    # (bucket, chunk_start, nchunks, end_of_half0)
    for hh in range(2):
        glo, ghi = hh * GH, min((hh + 1) * GH, ngrp)
        for bb in range(W):
            c0 = len(chunks)
            for g in range(glo, ghi):
                cell_off2[bb, g] = off
                off += int(budget[bb, g])
                nch = int(budget[bb, g]) // 128
                for j in range(nch):
                    chunks.append([(g, j == 0, j == nch - 1, bb == 0)])
            done = 0
            nb = len(chunks) - c0
            while done < nb:
                k = min(cfg["CALLCH"], nb - done)
                done += k
                calls.append((bb, c0 + done - k, k,
                              hh == 0 and bb == W - 1 and done >= nb))
    assert off == nslots
    # per-core slot data; drel stores NEGATED dst_rel (ACT Square bias), pad -> +1
    sub_qidx, sub_drel = [], []
    for c in range(W):
        b, q, dl, g = per_core[c]
        qpad = np.zeros(nslots, np.int64)
        dpad = np.full(nslots, -1, np.int64)
        cell = b * ngrp + g
        uniq, start_idx, counts = np.unique(cell, return_index=True, return_counts=True)
        for u, s0, k in zip(uniq, start_idx, counts):
            sl = cell_off2[u // ngrp, u % ngrp] + np.arange(k)
            qpad[sl] = q[s0:s0 + k]
            dpad[sl] = dl[s0:s0 + k] - (u % ngrp) * GRPW
        sub_qidx.append(_pack16(qpad))
        sub_drel.append((-dpad.reshape(-1, 128).T).astype(np.float32))  # [128, nchunks]
    plan["sub_nslots"] = nslots
    plan["sub_nops"] = nchunks
    plan["sub_qidx"] = sub_qidx
    plan["sub_drel"] = sub_drel
    plan["sub_chunks"] = chunks
    plan["sub_calls"] = calls
    plan["sub_ngrp"] = ngrp
    plan["sub_GH"] = GH

    # ---------- glob phase edge plan ----------
    gsrc, gdst = np.asarray(edge_index[0]), np.asarray(edge_index[1])
    gowner = gdst // n_glob
    gdst_local = gdst % n_glob
    gsrc_row = (gsrc // n_glob) * S_glob + (gsrc % n_glob)
    n_tiles_glob = S_glob // 128
    per_core_g = []
    CtG = np.zeros(n_tiles_glob, np.int64)
    for c in range(W):
        m = gowner == c
        sc, dl = gsrc_row[m], gdst_local[m]
        order = np.argsort(dl, kind="stable")
        sc, dl = sc[order], dl[order]
        per_core_g.append((sc, dl))
        gcnt = np.bincount(dl // 128, minlength=n_tiles_glob)
        CtG = np.maximum(CtG, (gcnt + 127) // 128)
    CtG = np.maximum(CtG, 1)
    plan["glob_Ct"] = CtG
    TOTCHG = int(CtG.sum())
    plan["glob_TOTCH"] = TOTCHG
    tile_slot_off_g = np.zeros(n_tiles_glob + 1, np.int64)
    tile_slot_off_g[1:] = np.cumsum(CtG * 128)
    gg_idx_cores, Sg_cores = [], []
    for c in range(W):
        sc, dl = per_core_g[c]
        g2 = np.zeros(TOTCHG * 128, np.int64)
        S = np.zeros((TOTCHG * 128, 128), BF)
        tl = dl // 128
        for t in range(n_tiles_glob):
            mt = tl == t
            k = int(mt.sum())
            slots = tile_slot_off_g[t] + np.arange(k)
            g2[slots] = sc[mt]
            S[slots, dl[mt] % 128] = BF(1.0)
        gg_idx_cores.append(_pack16(g2))
        Sg_cores.append(S)
    plan["glob_g2_idx"] = gg_idx_cores
    plan["glob_S"] = [S.reshape(TOTCHG, 128, 128).transpose(1, 0, 2).reshape(128, TOTCHG * 128)
                      for S in Sg_cores]

    # ---------- atom encode (host) ----------
    plan["aid_ids"] = np.asarray(x)[np.asarray(sub_node_map)]

    # ---------- phase boundary (roots) ----------
    tb_arr = np.asarray(target_batch)
    ri = np.asarray(root_idx)
    order = np.argsort(tb_arr, kind="stable")
    assert (np.bincount(tb_arr, minlength=N) == 2).all(), "need exactly 2 roots/node"
    r_sorted = ri[order].reshape(N, 2)
    lp_order = order.reshape(N, 2)
    plan["r0"], plan["r4"], plan["lp_sel"] = [], [], []
    for c in range(W):
        r = r_sorted[c * n_glob:(c + 1) * n_glob]
        lo = c * n_sub
        assert ((r >= lo) & (r < lo + n_sub)).all(), "roots must be core-local"
        r0 = np.zeros(S_glob, np.int64)
        r4 = np.zeros(S_glob, np.int64)
        r0[:n_glob] = r[:, 0] - lo
        r4[:n_glob] = r[:, 1] - lo
        plan["r0"].append(_pack16(r0))
        plan["r4"].append(_pack16(r4))
        plan["lp_sel"].append(lp_order[c * n_glob:(c + 1) * n_glob])

    # ---------- readout ----------
    b_arr = np.asarray(batch)
    plan["Sg"] = []
    for c in range(W):
        Srd = np.zeros((S_glob, cfg["G"]), BF)
        ids = b_arr[c * n_glob:(c + 1) * n_glob]
        Srd[np.arange(n_glob), ids] = BF(1.0)
        nt = S_glob // 128
        plan["Sg"].append(Srd.reshape(nt, 128, cfg["G"]).transpose(1, 0, 2).reshape(128, nt * cfg["G"]))
    return plan


def _install_queue_aware_lanes():
    """Make Tile's DMASW lane assignment queue-aware: lane = queue*2 + rr."""
    import concourse.tile_sem_assignment as tsa
    if getattr(tsa, "_qaware_installed", False):
        return
    orig = tsa.TileClockTick._assign_tick
    import concourse.mybir as mb

    def patched(self, inst):
        qn = getattr(inst, "queue_num", None)
        if (qn is not None and inst.engine == mb.EngineType.Pool
                and isinstance(inst, tsa.DMAInst)
                and self.swdge_sem_count == 8):
            rr_map = getattr(self, "_q_rr", None)
            if rr_map is None:
                rr_map = self._q_rr = {}
            sub = rr_map.get(qn, 0)
            rr_map[qn] = (sub + 1) % 2
            lane = qn * 2 + sub
            save = self.next_sw_dma_idx
            self.next_sw_dma_idx = lane
            try:
                return orig(self, inst)
            finally:
                self.next_sw_dma_idx = save
        return orig(self, inst)

    tsa.TileClockTick._assign_tick = patched
    tsa._qaware_installed = True


def build_graph(plan):
    from concourse import bass, mybir, bacc
    import concourse.tile as tile

    cfg = plan["cfg"]
    W, H, L = cfg["W"], cfg["H"], cfg["L"]
    n_sub, S_sub = cfg["n_sub"], cfg["S_sub"]
    n_glob, S_glob = cfg["n_glob"], cfg["S_glob"]
    G = cfg["G"]
    GRPW = cfg["GRPW"]
    BF16 = mybir.dt.bfloat16
    FP16 = mybir.dt.float16
    F32 = mybir.dt.float32
    I16 = mybir.dt.int16
    AF = mybir.ActivationFunctionType
    OP = mybir.AluOpType
    GRP = cfg["GRP"]
    ngrp = plan["sub_ngrp"]
    QACT, QCYC = cfg["QACT"], cfg["QCYC"]

    _install_queue_aware_lanes()
    nc = bacc.Bacc("TRN2", target_bir_lowering=False, debug=False, num_devices=W,
                   num_swdge_queues=4)

    def inp(name, shape, dt):
        return nc.dram_tensor(name, shape, dt, kind="ExternalInput")

    TOTCHG = plan["glob_TOTCH"]
    nslots, nops = plan["sub_nslots"], plan["sub_nops"]
    t_qidx = inp("qidx", [128, nslots // 16], I16)
    t_drel = inp("drel", [128, nops], F32)
    t_iota = inp("iota16", [128, GRPW], FP16)
    t_iotan = inp("iotan", [128, GRPW], BF16)
    t_gg = inp("ggidx", [128, TOTCHG * 8], I16)
    t_rep0 = inp("rep0", [W * S_sub, H], BF16)
    t_h0T = inp("h0T", [128, S_sub], BF16)
    t_r0 = inp("r0idx", [128, S_glob // 16], I16)
    t_r4 = inp("r4idx", [128, S_glob // 16], I16)
    t_Sglob = inp("Sglob", [128, TOTCHG * 128], BF16)
    t_Srd = inp("Srd", [128, (S_glob // 128) * G], BF16)
    t_idn_bf = inp("idnbf", [128, 128], BF16)
    t_idn_f = inp("idnf", [128, 128], F32)
    t_W1s = inp("W1s", [L, H, H], BF16)
    t_W2s = inp("W2s", [L, H, H], BF16)
    t_W1g = inp("W1g", [L, H, H], BF16)
    t_W2g = inp("W2g", [L, H, H], BF16)
    t_vecs = inp("vecs", [128, 10 * L], F32)
    t_lp = inp("lp", [S_glob, 2], F32)
    t_out = nc.dram_tensor("out", [G, H], F32, kind="ExternalOutput")

    rep_sub = nc.dram_tensor("rep_sub", [W * S_sub, H], BF16, addr_space="Shared")
    rep_glob = nc.dram_tensor("rep_glob", [W * S_glob, H], BF16, addr_space="Shared")
    hown_sub = nc.dram_tensor("hown_sub", [S_sub, H], BF16)
    hown_glob = nc.dram_tensor("hown_glob", [S_glob, H], BF16)
    ar_in = nc.dram_tensor("ar_in", [128, 2], F32)
    ar_out = nc.dram_tensor("ar_out", [128, 2], F32, addr_space="Shared")
    rd_in = nc.dram_tensor("rd_in", [128, G], F32)
    rd_out = nc.dram_tensor("rd_out", [128, G], F32, addr_space="Shared")

    RG = [list(range(W))]
    _qrr = [0]

    def nextq():
        q = (_qrr[0] % 8) // 2
        _qrr[0] += 1
        return q

    CtG = plan["glob_Ct"]
    chunks_meta = plan["sub_chunks"]
    calls_meta = plan["sub_calls"]
    GH = plan["sub_GH"]
    SPLITC = GH * GRPW

    with tile.TileContext(nc) as tc:
        with (
            tc.tile_pool(name="const", bufs=1) as constp,
            tc.tile_pool(name="xs", bufs=3) as xsp,
            tc.tile_pool(name="sgen", bufs=2) as sgp,
            tc.tile_pool(name="zz", bufs=3) as zzp,
            tc.tile_pool(name="small", bufs=2) as smp,
            tc.tile_pool(name="stg", bufs=2) as stgp,
            tc.tile_pool(name="psA", bufs=2, space="PSUM") as psA,
            tc.tile_pool(name="psM", bufs=1, space="PSUM") as psM,
            tc.tile_pool(name="psT", bufs=2, space="PSUM") as psT,
        ):
            # ---- constants resident ----
            vecs = constp.tile([128, 10 * L], F32)
            nc.sync.dma_start(vecs[:], t_vecs[:])
            W1s = constp.tile([128, L * H], BF16)
            W2s = constp.tile([128, L * H], BF16)
            W1g = constp.tile([128, L * H], BF16)
            W2g = constp.tile([128, L * H], BF16)
            for l in range(L):
                nc.sync.dma_start(W1s[:, l * H:(l + 1) * H], t_W1s[l])
                nc.sync.dma_start(W2s[:, l * H:(l + 1) * H], t_W2s[l])
                nc.sync.dma_start(W1g[:, l * H:(l + 1) * H], t_W1g[l])
                nc.sync.dma_start(W2g[:, l * H:(l + 1) * H], t_W2g[l])
            idn = constp.tile([128, 128], BF16, tag="idn")
            nc.sync.dma_start(idn[:], t_idn_bf[:])
            qidx_sb = constp.tile([128, nslots // 16], I16, tag="qidx")
            nc.sync.dma_start(qidx_sb[:], t_qidx[:])
            drel_sb = constp.tile([128, nops], F32, tag="drel")
            nc.sync.dma_start(drel_sb[:], t_drel[:])
            iota_sb = constp.tile([128, GRPW], FP16, tag="iota")
            nc.sync.dma_start(iota_sb[:], t_iota[:])
            iotan_sb = constp.tile([128, GRPW], BF16, tag="iotan")
            nc.sync.dma_start(iotan_sb[:], t_iotan[:])
            # zero tiles: let DVE ops that overlap gather descgen use 2-input
            # op classes (tensor_tensor / scalar_tensor_tensor), which never
            # grab the shared SBUF port pair that SWDGE descriptor generation
            # needs. tensor_scalar/copy would block GpSimd for their duration.
            zero_b = constp.tile([128, GRPW], BF16, tag="zerob")
            nc.vector.memset(zero_b[:], 0.0)
            zero_f = constp.tile([128, 512], F32, tag="zerof")
            nc.vector.memset(zero_f[:], 0.0)

            def vcol(phase, l, j):
                return vecs[:, (phase * 5 * L + l * 5 + j):(phase * 5 * L + l * 5 + j) + 1]

            # shared BN scalar computation (stats -> mu tile) ---------------
            def bn_head(phase, l, stats, n_real):
                ngr = (n_real + 511) // 512
                mv = smp.tile([128, 2], F32, tag="mv")
                nc.vector.bn_aggr(mv[:], stats[:, :ngr * 6])
                sin = smp.tile([128, 2], F32, tag="sin")
                nc.vector.tensor_tensor(sin[:, 1:2], mv[:, 0:1], mv[:, 0:1], op=OP.mult)
                nc.vector.tensor_tensor(sin[:, 1:2], sin[:, 1:2], mv[:, 1:2], op=OP.add)
                nc.vector.tensor_copy(sin[:, 0:1], mv[:, 0:1])
                nc.sync.dma_start(ar_in[:], sin[:])
                nc.gpsimd.collective_compute(
                    "AllReduce", OP.add, RG, [ar_in.ap().opt()], [ar_out.ap().opt()])
                sg = smp.tile([128, 2], F32, tag="sg")
                nc.sync.dma_start(sg[:], ar_out[:])
                mu = smp.tile([128, 4], F32, tag="mu")
                nc.scalar.mul(mu[:, 0:1], sg[:, 0:1], 1.0 / W)
                nc.scalar.mul(mu[:, 1:2], sg[:, 1:2], 1.0 / W)
                nc.vector.tensor_tensor(mu[:, 2:3], mu[:, 0:1], mu[:, 0:1], op=OP.mult)
                nc.vector.tensor_tensor(mu[:, 1:2], mu[:, 1:2], mu[:, 2:3], op=OP.subtract)
                nc.vector.tensor_scalar(mu[:, 1:2], mu[:, 1:2], float(cfg["BN_EPS"]), None,
                                        op0=OP.add)
                nc.scalar.activation(mu[:, 1:2], mu[:, 1:2], AF.Sqrt, bias=0.0, scale=1.0)
                nc.vector.reciprocal(mu[:, 1:2], mu[:, 1:2])
                nc.vector.tensor_tensor(mu[:, 2:3], vcol(phase, l, 2), mu[:, 1:2], op=OP.mult)
                nc.vector.tensor_tensor(mu[:, 3:4], mu[:, 0:1], mu[:, 2:3], op=OP.mult)
                nc.vector.tensor_tensor(mu[:, 3:4], vcol(phase, l, 3), mu[:, 3:4], op=OP.subtract)
                return mu

            # normalize z (SBUF, h^T layout) + residual into hT, write hown
            def bn_apply(mu, zread, hT, hown, Sp, n_tiles, readout=None):
                stgt = stgp.tile([128, 16, 128], BF16, tag="stg")
                stg_fill = 0
                stg_t0 = 0
                for g in range((Sp + 511) // 512):
                    base = g * 512
                    cols = min(512, Sp - base)
                    zt_, co_ = zread(g)
                    hn = zzp.tile([128, 512], BF16, tag="hn")
                    nc.scalar.activation(hn[:, :cols], zt_[:, co_:co_ + cols],
                                         AF.Identity, bias=mu[:, 3:4],
                                         scale=mu[:, 2:3])
                    nc.vector.tensor_tensor(hT[:, base:base + cols], hn[:, :cols],
                                            hT[:, base:base + cols], op=OP.add)
                    for q in range(cols // 128):
                        t = base // 128 + q
                        pt = psT.tile([128, 128], BF16, tag="tr")
                        nc.tensor.transpose(pt[:], hT[:, t * 128:(t + 1) * 128], idn[:])
                        nc.vector.tensor_copy(stgt[:, stg_fill, :], pt[:])
                        if readout is not None:
                            prd, srd_all = readout
                            nc.tensor.matmul(prd[:], stgt[:, stg_fill, :],
                                             srd_all[:, t * G:(t + 1) * G],
                                             start=(t == 0), stop=(t == n_tiles - 1))
                        stg_fill += 1
                        if stg_fill == 16 or t == n_tiles - 1:
                            nc.sync.dma_start(
                                hown.ap().rearrange("(c p) h -> p c h", p=128)[:, stg_t0:stg_t0 + stg_fill, :],
                                stgt[:, :stg_fill, :])
                            stg_t0 += stg_fill
                            stg_fill = 0
                            if t != n_tiles - 1:
                                stgt = stgp.tile([128, 16, 128], BF16, tag="stg")

            # ================= sub phase =================
            with tc.tile_pool(name="subres", bufs=1) as subres:
                hTs = subres.tile([128, S_sub], BF16, tag="hTs")
                nc.sync.dma_start(hTs[:], t_h0T[:])

                def gin_sub(l):
                    if l > 0:
                        nc.gpsimd.collective_compute(
                            "AllGather", OP.bypass, RG,
                            [hown_sub.ap().opt()], [rep_sub.ap().opt()])
                        rep_src = rep_sub
                    else:
                        rep_src = t_rep0
                    acc0 = subres.tile([128, SPLITC], BF16, tag="acc0")
                    acc1 = subres.tile([128, S_sub - SPLITC], BF16, tag="acc1")
                    stats = smp.tile([128, 64 * 6], F32, tag="stats")

                    def acc_at(g):
                        if g < GH:
                            return acc0, g * GRPW
                        return acc1, g * GRPW - SPLITC

                    def acc512(g):
                        co = g * 512
                        if co < SPLITC:
                            return acc0, co
                        return acc1, co - SPLITC

                    def epilogue(g0, g1):
                        for g in range(g0, g1):
                            cols = min(512, S_sub - g * 512)
                            at, co = acc512(g)
                            zg = zzp.tile([128, 512], BF16, tag="zg2")
                            nc.vector.scalar_tensor_tensor(
                                zg[:, :cols], hTs[:, g * 512:g * 512 + cols],
                                vcol(0, l, 4), at[:, co:co + cols],
                                op0=OP.mult, op1=OP.add)
                            pm = psM.tile([128, 512], F32, tag="m1")
                            nc.tensor.matmul(pm[:, :cols], W1s[:, l * H:(l + 1) * H],
                                             zg[:, :cols], start=True, stop=True)
                            z1 = zzp.tile([128, 512], BF16, tag="z1")
                            nc.scalar.activation(z1[:, :cols], pm[:, :cols], AF.Relu,
                                                 bias=vcol(0, l, 0), scale=1.0)
                            pm2 = psM.tile([128, 512], F32, tag="m2")
                            nc.tensor.matmul(pm2[:, :cols], W2s[:, l * H:(l + 1) * H],
                                             z1[:, :cols], start=True, stop=True)
                            nc.vector.tensor_scalar(at[:, co:co + cols], pm2[:, :cols],
                                                    vcol(0, l, 1), None, op0=OP.add)
                            realc = min(512, max(0, n_sub - g * 512))
                            if realc > 0:
                                nc.vector.bn_stats(stats[:, g * 6:(g + 1) * 6],
                                                   at[:, co:co + realc])

                    op_i = 0
                    ps_of = {}
                    for (bb, c0, k, h0end) in calls_meta:
                        xt = xsp.tile([128, cfg["CALLCH"], H], BF16, tag="x")
                        n = k * 128
                        nc.gpsimd.dma_gather(
                            xt[:, :k, :], rep_src[bb * S_sub:(bb + 1) * S_sub, :],
                            qidx_sb[:, (c0 * 128) // 16:(c0 * 128 + n) // 16], n, n, H,
                            single_packet=False, queue_num=nextq())
                        # S-gen: quads on Scalar (Square+fused Relu), rest on
                        # DVE (is_equal against negated iota).
                        St4_of = {}
                        for jq in range(0, k, 4):
                            kq = min(4, k - jq)
                            if (jq // 4) % QCYC < QACT:
                                sq4 = sgp.tile([128, 4 * GRPW], BF16, tag="sgq")
                                for t in range(kq):
                                    nc.scalar.activation(
                                        sq4[:, t * GRPW:(t + 1) * GRPW], iota_sb[:],
                                        AF.Square,
                                        bias=drel_sb[:, op_i + jq + t:op_i + jq + t + 1],
                                        scale=1.0)
                                St4 = sgp.tile([128, 4 * GRPW], BF16, tag="sgen")
                                nc.scalar.activation(St4[:, :kq * GRPW], sq4[:, :kq * GRPW],
                                                     AF.Relu, bias=1.0, scale=-1.0)
                                for t in range(kq):
                                    St4_of[jq + t] = (St4, t * GRPW)
                            else:
                                for t in range(kq):
                                    sd = sgp.tile([128, GRPW], BF16, tag="sgd", bufs=6)
                                    nc.vector.scalar_tensor_tensor(
                                        sd[:], iotan_sb[:],
                                        drel_sb[:, op_i + jq + t:op_i + jq + t + 1],
                                        zero_b[:], op0=OP.is_equal, op1=OP.add)
                                    St4_of[jq + t] = (sd, 0)
                        for j in range(k):
                            ops = chunks_meta[c0 + j]
                            for (g, first, stop, gfirst) in ops:
                                St4, sco = St4_of[j]
                                if first:
                                    ps_of[g] = psA.tile([128, GRPW], F32, tag="agg2",
                                                        name=f"agg{g % 2}")
                                cols = min(GRPW, S_sub - g * GRPW)
                                nc.tensor.matmul(ps_of[g][:, :cols], xt[:, j, :],
                                                 St4[:, sco:sco + cols],
                                                 start=first, stop=stop)
                                if stop:
                                    pt = ps_of.pop(g)
                                    at, co = acc_at(g)
                                    if gfirst:
                                        nc.vector.tensor_tensor(
                                            at[:, co:co + cols], pt[:, :cols],
                                            zero_f[:, :cols], op=OP.add)
                                    else:
                                        nc.vector.tensor_tensor(
                                            at[:, co:co + cols],
                                            at[:, co:co + cols],
                                            pt[:, :cols], op=OP.add)
                                op_i += 1
                        if h0end:
                            epilogue(0, SPLITC // 512)
                    epilogue(SPLITC // 512, (S_sub + 511) // 512)
                    mu = bn_head(0, l, stats, n_sub)
                    bn_apply(mu, acc512, hTs, hown_sub, S_sub, S_sub // 128)

                for l in range(L):
                    gin_sub(l)

            # ================= phase boundary =================
            with (
                tc.tile_pool(name="gres", bufs=1) as gres,
                tc.tile_pool(name="seg", bufs=2) as segp,
                tc.tile_pool(name="idx", bufs=1) as idxp,
                tc.tile_pool(name="gxs", bufs=3) as gxsp,
            ):
                hTg = gres.tile([128, S_glob], BF16, tag="hTg")
                gg_sb = gres.tile([128, TOTCHG * 8], I16, tag="ggix")
                nc.sync.dma_start(gg_sb[:], t_gg[:])

                r0_sb = idxp.tile([128, S_glob // 16], I16, tag="r0")
                r4_sb = idxp.tile([128, S_glob // 16], I16, tag="r4")
                nc.sync.dma_start(r0_sb[:], t_r0[:])
                nc.sync.dma_start(r4_sb[:], t_r4[:])
                r0b = segp.tile([128, S_glob // 128, H], BF16, tag="seg")
                r4b = segp.tile([128, S_glob // 128, H], BF16, tag="seg")
                nc.gpsimd.dma_gather(r0b[:], hown_sub[:], r0_sb[:], S_glob, S_glob, H,
                                     single_packet=False, queue_num=nextq())
                nc.gpsimd.dma_gather(r4b[:], hown_sub[:], r4_sb[:], S_glob, S_glob, H,
                                     single_packet=False, queue_num=nextq())
                inv_temp = 1.0 / float(cfg["TEMP"])
                ntg = S_glob // 128
                lpt = smp.tile([128, 2, ntg], F32, tag="lpt")
                nc.sync.dma_start(lpt[:],
                                  t_lp.ap().rearrange("(c p) k -> p k c", p=128))
                dd = smp.tile([128, 2, ntg], F32, tag="d")
                nc.vector.tensor_tensor(dd[:, 0:1, :], lpt[:, 0:1, :],
                                        lpt[:, 1:2, :], op=OP.subtract)
                nc.scalar.activation(dd[:, 0:1, :], dd[:, 0:1, :], AF.Sigmoid,
                                     bias=0.0, scale=inv_temp)
                nc.vector.tensor_scalar(dd[:, 1:2, :], dd[:, 0:1, :], -1.0, 1.0,
                                        op0=OP.mult, op1=OP.add)
                hball = segp.tile([128, ntg, H], BF16, tag="hball")
                for t in range(ntg):
                    hb = hball[:, t, :]
                    nc.vector.tensor_scalar(hb, r0b[:, t, :], dd[:, 0:1, t:t + 1],
                                            None, op0=OP.mult)
                    nc.vector.scalar_tensor_tensor(hb, r4b[:, t, :],
                                                   dd[:, 1:2, t:t + 1], hb,
                                                   op0=OP.mult, op1=OP.add)
                    pt = psT.tile([128, 128], BF16, tag="tr")
                    nc.tensor.transpose(pt[:], hb, idn[:])
                    nc.vector.tensor_copy(hTg[:, t * 128:(t + 1) * 128], pt[:])
                nc.sync.dma_start(
                    hown_glob.ap().rearrange("(c p) h -> p c h", p=128),
                    hball[:])

                # ================= glob phase =================
                def gin_glob(l):
                    Sp, n_real, rep, hown = S_glob, n_glob, rep_glob, hown_glob
                    n_tiles = Sp // 128
                    nc.gpsimd.collective_compute(
                        "AllGather", OP.bypass, RG, [hown.ap().opt()], [rep.ap().opt()])
                    z2 = gres.tile([128, S_glob], BF16, tag="z2g")
                    stats = smp.tile([128, 64 * 6], F32, tag="stats")
                    chunks = []
                    for t in range(n_tiles):
                        for j in range(int(CtG[t])):
                            chunks.append((t, j == 0, j == int(CtG[t]) - 1))
                    psum_of = {}
                    groups = [chunks[i:i + GRP] for i in range(0, len(chunks), GRP)]
                    ch_base = 0
                    zgrp_tiles = {}

                    def run_mlp(g):
                        zg = zgrp_tiles.pop(g)
                        cols = min(512, Sp - g * 512)
                        pm = psM.tile([128, 512], F32, tag="m1")
                        nc.tensor.matmul(pm[:, :cols], W1g[:, l * H:(l + 1) * H], zg[:, :cols],
                                         start=True, stop=True)
                        z1 = zzp.tile([128, 512], BF16, tag="z1")
                        nc.scalar.activation(z1[:, :cols], pm[:, :cols], AF.Relu,
                                             bias=vcol(1, l, 0), scale=1.0)
                        pm2 = psM.tile([128, 512], F32, tag="m2")
                        nc.tensor.matmul(pm2[:, :cols], W2g[:, l * H:(l + 1) * H], z1[:, :cols],
                                         start=True, stop=True)
                        nc.vector.tensor_scalar(z2[:, g * 512:g * 512 + cols], pm2[:, :cols],
                                                vcol(1, l, 1), None, op0=OP.add)
                        realc = min(512, max(0, n_real - g * 512))
                        if realc > 0:
                            nc.vector.bn_stats(stats[:, g * 6:(g + 1) * 6],
                                               z2[:, g * 512:g * 512 + realc])

                    for gci, grp in enumerate(groups):
                        nch = len(grp)
                        xt = gxsp.tile([128, GRP, H], BF16, tag="gx")
                        n = nch * 128
                        nc.gpsimd.dma_gather(
                            xt[:, :nch, :], rep[:, :],
                            gg_sb[:, ch_base * 8:(ch_base + nch) * 8],
                            n, n, H, single_packet=False, queue_num=nextq())
                        st = gxsp.tile([128, GRP * 128], BF16, tag="gs")
                        nc.sync.dma_start(st[:, :n], t_Sglob[:, ch_base * 128:ch_base * 128 + n])
                        for j, (t, first, lastc) in enumerate(grp):
                            if first:
                                psum_of[t] = psA.tile([128, 512], F32, tag="agg",
                                                      name=f"aggg{t % 8}")
                            nc.tensor.matmul(psum_of[t][:, :128], xt[:, j, :],
                                             st[:, j * 128:(j + 1) * 128],
                                             start=first, stop=lastc)
                            if lastc:
                                g = (t * 128) // 512
                                if g not in zgrp_tiles:
                                    zgrp_tiles[g] = zzp.tile([128, 512], BF16, tag="zg",
                                                             name=f"zg{g % 4}")
                                cc = t * 128 - g * 512
                                pt = psum_of.pop(t)
                                nc.vector.scalar_tensor_tensor(
                                    zgrp_tiles[g][:, cc:cc + 128],
                                    hTg[:, t * 128:(t + 1) * 128],
                                    vcol(1, l, 4), pt[:, :128],
                                    op0=OP.mult, op1=OP.add)
                                if (t * 128 + 128) % 512 == 0 or t == n_tiles - 1:
                                    run_mlp(g)
                        ch_base += nch
                    mu = bn_head(1, l, stats, n_glob)

                    def zread(g):
                        return z2, g * 512
                    if l == L - 1:
                        prd = psM.tile([128, G], F32, tag="m1")
                        srd_all = gres.tile([128, (S_glob // 128) * G], BF16, tag="srda")
                        nc.sync.dma_start(srd_all[:], t_Srd[:])
                        bn_apply(mu, zread, hTg, hown_glob, S_glob, n_tiles,
                                 readout=(prd, srd_all))
                    else:
                        bn_apply(mu, zread, hTg, hown_glob, S_glob, n_tiles)
                    return prd if l == L - 1 else None

                for l in range(L):
                    prd = gin_glob(l)

                # ---- readout ----
                rd_sb = gres.tile([128, G], F32, tag="rdsb")
                nc.vector.tensor_copy(rd_sb[:], prd[:])
                nc.sync.dma_start(rd_in[:], rd_sb[:])
                nc.gpsimd.collective_compute(
                    "AllReduce", OP.add, RG, [rd_in.ap().opt()], [rd_out.ap().opt()])
                rd2 = gres.tile([128, G], F32, tag="rd2")
                nc.sync.dma_start(rd2[:], rd_out[:])
                idf = gres.tile([128, 128], F32, tag="idf")
                nc.sync.dma_start(idf[:], t_idn_f[:])
                for g in range((G + 127) // 128):
                    cols = min(128, G - g * 128)
                    pt = psA.tile([128, 512], F32, tag="agg")
                    nc.tensor.transpose(pt[:cols, :128], rd2[:, g * 128:g * 128 + cols], idf[:])
                    ot = gres.tile([128, 128], F32, tag="ot")
                    nc.vector.tensor_copy(ot[:cols, :], pt[:cols, :128])
                    nc.sync.dma_start(t_out[g * 128:g * 128 + cols, :], ot[:cols, :])

    nc.compile()
    return nc


def build_inmaps(plan, weights):
    cfg = plan["cfg"]
    W, H, L = cfg["W"], cfg["H"], cfg["L"]
    n_glob, S_glob = cfg["n_glob"], cfg["S_glob"]
    lp = np.asarray(weights["log_probs"], np.float32)
    maps = []
    vecs = np.zeros((128, 10 * L), np.float32)
    for ph, pre in ((0, "sub"), (1, "glob")):
        for l in range(L):
            base = ph * 5 * L + l * 5
            vecs[:, base + 0] = np.asarray(weights[f"{pre}_b1"][l], np.float32)
            vecs[:, base + 1] = np.asarray(weights[f"{pre}_b2"][l], np.float32)
            vecs[:, base + 2] = np.asarray(weights[f"{pre}_gamma"][l], np.float32)
            vecs[:, base + 3] = np.asarray(weights[f"{pre}_beta"][l], np.float32)
            vecs[:, base + 4] = 1.0 + np.float32(weights[f"{pre}_eps"][l])
    idn = np.eye(128)
    iota = np.tile(np.arange(cfg["GRPW"], dtype=np.float32), (128, 1)).astype(F16)
    iotan = np.tile(-np.arange(cfg["GRPW"], dtype=np.float32), (128, 1)).astype(BF)
    common = {
        "W1s": np.asarray(weights["sub_W1"], np.float32).astype(BF),
        "W2s": np.asarray(weights["sub_W2"], np.float32).astype(BF),
        "W1g": np.asarray(weights["glob_W1"], np.float32).astype(BF),
        "W2g": np.asarray(weights["glob_W2"], np.float32).astype(BF),
        "vecs": vecs,
        "idnbf": idn.astype(BF),
        "idnf": idn.astype(np.float32),
        "iota16": iota,
        "iotan": iotan,
    }
    atom_bf = np.asarray(weights["atom_table"], np.float32).astype(BF)
    n_sub, S_sub = cfg["n_sub"], cfg["S_sub"]
    h0_all = atom_bf[plan["aid_ids"]]
    rep0 = np.zeros((W * S_sub, H), BF)
    for b in range(W):
        rep0[b * S_sub:b * S_sub + n_sub] = h0_all[b * n_sub:(b + 1) * n_sub]
    common["rep0"] = rep0
    for c in range(W):
        h0T = np.zeros((128, S_sub), BF)
        h0T[:, :n_sub] = h0_all[c * n_sub:(c + 1) * n_sub].T
        lpc = np.zeros((S_glob, 2), np.float32)
        lpc[:n_glob] = lp[plan["lp_sel"][c]]
        m = dict(common)
        m.update({
            "qidx": plan["sub_qidx"][c],
            "drel": plan["sub_drel"][c],
            "ggidx": plan["glob_g2_idx"][c],
            "h0T": h0T,
            "r0idx": plan["r0"][c],
            "r4idx": plan["r4"][c],
            "Sglob": plan["glob_S"][c],
            "Srd": plan["Sg"][c],
            "lp": lpc,
        })
        maps.append(m)
    return maps


def kernel(**inputs):
    import numpy as np
    cfg = dict(DEF_CFG)
    inp = {k: np.asarray(v) for k, v in inputs.items()}
    plan = build_plan(cfg, inp["x"], inp["edge_index"], inp["sub_node_map"],
                      inp["sub_edge_index"], inp["root_idx"], inp["target_batch"],
                      inp["batch"])
    nc = build_graph(plan)
    maps = build_inmaps(plan, inp)
    from concourse import bass_utils
    res = bass_utils.run_bass_kernel_spmd(nc, maps, core_ids=list(range(cfg["W"])),
                                          trace=False)
    return np.asarray(res.results[0]["out"], np.float32)
